# revision 25
# baseline (speedup 1.0000x reference)
"""Trainium2 Bass kernel for the 2-layer GAT block (nn_GATblock_58282706206740).

Strategy (8 NeuronCores, SPMD, dst-sharded):
  - Edges (incl. self-loops) sharded by destination-node range: core c owns
    dst nodes [1250c, 1250(c+1)), split into 10 blocks of 125. Per-(core,
    block) edge lists padded to a common per-block tile count (max over
    cores) of 128-edge tiles, so one program serves all cores.
  - Phase A (replicated, bf16): node table rows [asrc1(5)|adst1(5)|h1(320)]
    written to DRAM (768B rows); per-edge rows arrive via gpsimd dma_gather.
  - dma_gather descriptor generation is the machine's serial bottleneck
    (~8.4 ns/idx on one Q7 core pair); gathers round-robin over 4 SWDGE
    queues so 4 desc-gens+transfers run concurrently (~4x).
  - Scores e = a_src[src]+a_dst[dst] are summed in PSUM by two accumulating
    PE matmuls per tile (a_dst expands via the fp8 ST one-hot; a_src copies
    in via an identity matmul). leaky-relu/exp run chunk-batched on the
    Scalar engine (Prelu/Exp share one activation table); exp*h is one
    chunk-batched DVE multiply in bf16.
  - Segment softmax-sum + message aggregation is one fused PE matmul per
    tile with rhs [exp | exp*h] (bf16) against the fp8 S one-hot (0/1 is
    exact in fp8; fp8 lhsT pairs with bf16 rhs, verified exact on HW).
    Softmax skips the max subtraction (scores provably small) and keeps
    the reference's +1e-16 denominator epsilon.
  - Between layers one AllGather exchanges the bf16 [h2 | a_src2] table.
"""
import sys

sys.path.insert(0, "/opt/trn_rl_repo")

import ml_dtypes
import numpy as np

N_NODES = 10000
N_CORES = 8
NPC = N_NODES // N_CORES          # 1250
B_BLOCKS = 10
NPB = NPC // B_BLOCKS             # 125
TILE_E = 128
CHUNK = 8                         # tiles per gather call (1024 idx)
EPS = 1e-16
NEG_SLOPE = 0.2
F0, F1, F2, H1, C1 = 128, 320, 64, 5, 64
ROW1 = 384                        # [asrc1(5) | adst1(5) | h1(320) | pad]
ROW2 = 128                        # [h2(64) | asrc2(1) | pad]


def _build_partition(edge_index):
    src = np.concatenate([edge_index[0].astype(np.int64),
                          np.arange(N_NODES, dtype=np.int64)])
    dst = np.concatenate([edge_index[1].astype(np.int64),
                          np.arange(N_NODES, dtype=np.int64)])
    core = dst // NPC
    block = (dst % NPC) // NPB
    col = dst % NPB

    cnt = np.zeros((N_CORES, B_BLOCKS), dtype=np.int64)
    np.add.at(cnt, (core, block), 1)
    T_b = np.ceil(cnt.max(axis=0) / TILE_E).astype(np.int64)
    tile_ofs = np.concatenate([[0], np.cumsum(T_b)])
    Ttot = int(tile_ofs[-1])
    Epad = Ttot * TILE_E

    src_sl = np.zeros((N_CORES, Epad), dtype=np.int64)
    col_sl = np.full((N_CORES, Epad), 200.0, dtype=np.float32)
    order = np.lexsort((dst, core * B_BLOCKS + block))
    s_src, s_core, s_block, s_col = src[order], core[order], block[order], col[order]
    idx = 0
    for c in range(N_CORES):
        for b in range(B_BLOCKS):
            n = int(cnt[c, b])
            base = int(tile_ofs[b]) * TILE_E
            sl = slice(idx, idx + n)
            assert np.all(s_core[sl] == c) and np.all(s_block[sl] == b)
            src_sl[c, base:base + n] = s_src[sl]
            col_sl[c, base:base + n] = s_col[sl]
            idx += n
    assert idx == len(src)
    return src_sl, col_sl, tile_ofs, Ttot, Epad


def _wrap_idx16(idx):
    a = idx.astype(np.int16).reshape(-1, 16).T
    return np.tile(a, (8, 1))


def _host_prep(inputs):
    x = np.asarray(inputs["x"], dtype=np.float32)
    W1 = np.asarray(inputs["W1"], dtype=np.float32)
    att_src1 = np.asarray(inputs["att_src1"], dtype=np.float32)
    att_dst1 = np.asarray(inputs["att_dst1"], dtype=np.float32)
    b1 = np.asarray(inputs["b1"], dtype=np.float32)
    W2 = np.asarray(inputs["W2"], dtype=np.float32)
    att_src2 = np.asarray(inputs["att_src2"], dtype=np.float32)
    att_dst2 = np.asarray(inputs["att_dst2"], dtype=np.float32)
    b2 = np.asarray(inputs["b2"], dtype=np.float32)
    ei = np.asarray(inputs["edge_index"])

    src_sl, col_sl, tile_ofs, Ttot, Epad = _build_partition(ei)

    bf16 = ml_dtypes.bfloat16
    fp8 = ml_dtypes.float8_e4m3fn

    # W1cat = [W1@Asrc(5) | W1@Adst(5) | W1(320)] -> node row = x @ W1cat
    W1Asrc = np.stack([W1[:, 64 * h:64 * h + 64] @ att_src1[h] for h in range(H1)], axis=1)
    W1Adst = np.stack([W1[:, 64 * h:64 * h + 64] @ att_dst1[h] for h in range(H1)], axis=1)
    W1cat = np.concatenate([W1Asrc, W1Adst, W1], axis=1).astype(bf16)  # [128, 330]

    # W2cat tiles: [128, 3, 66] = [W2 | W2@asrc2 | W2@adst2] zero-padded
    W2c = np.concatenate([W2, (W2 @ att_src2[0])[:, None],
                          (W2 @ att_dst2[0])[:, None]], axis=1)  # [320, 66]
    W2p = np.zeros((384, 66), dtype=np.float32)
    W2p[:320] = W2c
    W2cat = np.ascontiguousarray(W2p.reshape(3, 128, 66).transpose(1, 0, 2)).astype(bf16)

    xT = np.ascontiguousarray(x.T)
    shared = dict(
        xT=xT.astype(bf16),                              # [128, 10000]
        W1cat=W1cat,
        W2cat=W2cat,
        ident=np.eye(128, dtype=np.float32).astype(bf16),
        b1rep=np.broadcast_to(b1, (128, F1)).copy(),
        b2rep=np.broadcast_to(b2, (128, F2)).copy(),
    )
    # h1 table row permutation: phase A writes groups of 4 node tiles as
    # [p, i] -> row 512g + 4p + i so each partition's 4 rows are contiguous
    # (3KB runs -> 4x fewer DMA descriptors). Tail nodes keep identity rows.
    n_full = (N_NODES // 512) * 512
    nn = np.arange(N_NODES, dtype=np.int64)
    perm1 = np.where(
        nn < n_full,
        (nn // 512) * 512 + (nn % 128) * 4 + (nn % 512) // 128,
        nn)

    d = np.arange(128, dtype=np.float32)
    per_core = []
    for c in range(N_CORES):
        colf = np.ascontiguousarray(col_sl[c].reshape(Ttot, TILE_E).T)  # [128, Ttot]
        S = (colf[:, :, None] == d[None, None, :])                      # [e,t,d]
        per_core.append(dict(
            src16=_wrap_idx16(perm1[src_sl[c]]),
            src16b=_wrap_idx16(src_sl[c]),
            Sb=np.ascontiguousarray(S).astype(fp8),
            STb=np.ascontiguousarray(np.transpose(S, (2, 1, 0))).astype(fp8),
            xTc=np.ascontiguousarray(xT[:, c * NPC:(c + 1) * NPC]).astype(bf16),
        ))
    return shared, per_core, tile_ofs, Ttot, Epad


def _build_program(tile_ofs, Ttot, Epad):
    import concourse.bacc as bacc
    import concourse.mybir as mybir
    from concourse import tile

    dt = mybir.dt
    F32 = dt.float32
    BF16 = dt.bfloat16
    FP8 = dt.float8e4
    AF = mybir.ActivationFunctionType
    OP = mybir.AluOpType

    B = B_BLOCKS
    tile_ofs = [int(v) for v in tile_ofs]
    block_of_tile = np.zeros(Ttot, dtype=np.int64)
    for b in range(B):
        block_of_tile[tile_ofs[b]:tile_ofs[b + 1]] = b
    n_chunks = (Ttot + CHUNK - 1) // CHUNK
    n_node_tiles = (N_NODES + 127) // 128

    nc = bacc.Bacc("TRN2", target_bir_lowering=False, debug=False,
                   num_devices=N_CORES, num_swdge_queues=4)

    xT_d = nc.dram_tensor("xT", [F0, N_NODES], BF16, kind="ExternalInput")
    W1c_d = nc.dram_tensor("W1cat", [F0, 2 * H1 + F1], BF16, kind="ExternalInput")
    W2c_d = nc.dram_tensor("W2cat", [128, 3, F2 + 2], BF16, kind="ExternalInput")
    ident_d = nc.dram_tensor("ident", [128, 128], BF16, kind="ExternalInput")
    b1_d = nc.dram_tensor("b1rep", [128, F1], F32, kind="ExternalInput")
    b2_d = nc.dram_tensor("b2rep", [128, F2], F32, kind="ExternalInput")
    src16_d = nc.dram_tensor("src16", [128, Epad // 16], dt.int16, kind="ExternalInput")
    src16b_d = nc.dram_tensor("src16b", [128, Epad // 16], dt.int16, kind="ExternalInput")
    S_d = nc.dram_tensor("Sb", [128, Ttot, 128], FP8, kind="ExternalInput")
    ST_d = nc.dram_tensor("STb", [128, Ttot, 128], FP8, kind="ExternalInput")
    xTc_d = nc.dram_tensor("xTc", [F0, NPC], BF16, kind="ExternalInput")
    out_d = nc.dram_tensor("out", [NPC, F2], F32, kind="ExternalOutput")

    with tile.TileContext(nc) as tc:
        with (
            tc.tile_pool(name="dram", bufs=1, space="DRAM") as dram,
            tc.tile_pool(name="const", bufs=1) as cpool,
        ):
            h1tab = dram.tile([N_NODES, ROW1], BF16)
            ag_in = dram.tile([NPC, F2 + 2], BF16)
            h2pack = dram.tile([N_NODES, F2 + 2], BF16, addr_space="Shared")
            h2tab = dram.tile([N_NODES, ROW2], BF16)

            W1cs = cpool.tile([F0, 2 * H1 + F1], BF16)
            nc.sync.dma_start(W1cs[:], W1c_d[:])
            W2cs = cpool.tile([128, 3, F2 + 2], BF16)
            nc.sync.dma_start(W2cs[:], W2c_d[:])
            idents = cpool.tile([128, 128], BF16)
            nc.sync.dma_start(idents[:], ident_d[:])
            b1s = cpool.tile([128, F1], F32)
            nc.sync.dma_start(b1s[:], b1_d[:])
            b2s = cpool.tile([128, F2], F32)
            nc.sync.dma_start(b2s[:], b2_d[:])
            src16 = cpool.tile([128, Epad // 16], dt.int16)
            nc.sync.dma_start(src16[:], src16_d[:])
            src16b = cpool.tile([128, Epad // 16], dt.int16)
            nc.sync.dma_start(src16b[:], src16b_d[:])
            S_sb = cpool.tile([128, Ttot, 128], FP8)
            nc.sync.dma_start(S_sb[:], S_d[:])
            ST_sb = cpool.tile([128, Ttot, 128], FP8)
            nc.sync.dma_start(ST_sb[:], ST_d[:])
            xTcs = cpool.tile([F0, NPC], BF16)
            nc.sync.dma_start(xTcs[:], xTc_d[:])
            adst1s = cpool.tile([128, B, H1], BF16)
            adst2s = cpool.tile([128, B, 1], BF16)

            # ---- phase A: node table + own-dst adst1 -----------------------
            with (
                tc.tile_pool(name="pAx", bufs=1) as pAx,
                tc.tile_pool(name="pA", bufs=3) as pA,
                tc.tile_pool(name="psA", bufs=3, space="PSUM") as psA,
            ):
                for b in range(B):
                    pa = psA.tile([128, H1], F32, tag="pa")
                    nc.tensor.matmul(pa[:NPB, :], xTcs[:, NPB * b:NPB * (b + 1)],
                                     W1cs[:, H1:2 * H1], start=True, stop=True)
                    nc.vector.tensor_copy(adst1s[:NPB, b, :], pa[:NPB, :])
                xTs = pAx.tile([F0, N_NODES], BF16)
                XCH = 2560
                for xo in range(0, N_NODES, XCH):
                    xw = min(XCH, N_NODES - xo)
                    nc.sync.dma_start(xTs[:, xo:xo + xw], xT_d[:, xo:xo + xw])
                GRP = 4
                for nt0 in range(0, n_node_tiles, GRP):
                    gn = min(GRP, n_node_tiles - nt0)
                    row = pA.tile([128, GRP, ROW1], BF16, tag="row")
                    for i in range(gn):
                        nt = nt0 + i
                        w = min(128, N_NODES - 128 * nt)
                        ph = psA.tile([128, 2 * H1 + F1], F32, tag="ph")
                        nc.tensor.matmul(ph[:w, :], xTs[:, 128 * nt:128 * nt + w],
                                         W1cs[:], start=True, stop=True)
                        if i % 2 == 0:
                            nc.vector.tensor_copy(row[:w, i, 0:2 * H1 + F1],
                                                  ph[:w, :])
                        else:
                            nc.scalar.activation(row[:w, i, 0:2 * H1 + F1],
                                                 ph[:w, :], AF.Copy)
                    if 128 * (nt0 + gn) <= n_node_tiles * 128 and gn == GRP:
                        # permuted rows: row 512g + 4p + i; [p, i] contiguous
                        dst = h1tab[512 * (nt0 // GRP):512 * (nt0 // GRP + 1), :]
                        dst = dst.rearrange("(p i) r -> p i r", i=GRP)
                        nc.sync.dma_start(dst, row[:])
                    else:
                        for i in range(gn):
                            nt = nt0 + i
                            w = min(128, N_NODES - 128 * nt)
                            nc.sync.dma_start(
                                h1tab[128 * nt:128 * nt + w, 0:2 * H1 + F1],
                                row[:w, i, 0:2 * H1 + F1])

            # ---- layer 1 edge sweep ---------------------------------------
            with (
                tc.tile_pool(name="gbuf", bufs=12) as gbuf,
                tc.tile_pool(name="sb1", bufs=4) as sb1,
                tc.tile_pool(name="ps_es", bufs=3, space="PSUM") as ps_es,
                tc.tile_pool(name="ps_u", bufs=2, space="PSUM") as ps_u,
                tc.tile_pool(name="ps_t", bufs=1, space="PSUM") as ps_t,
                tc.tile_pool(name="epi", bufs=2) as epi,
            ):
                usp = None
                for ch in range(n_chunks):
                    t0 = ch * CHUNK
                    tn = min(CHUNK, Ttot - t0)
                    g = gbuf.tile([128, CHUNK, ROW1], BF16, tag="g")
                    nc.gpsimd.dma_gather(
                        g[:, 0:tn, :], h1tab[:],
                        src16[:, t0 * 8:t0 * 8 + tn * 8],
                        num_idxs=tn * TILE_E, num_idxs_reg=tn * TILE_E,
                        elem_size=ROW1, queue_num=ch % 4)
                    esp = ps_es.tile([128, CHUNK, H1], F32, tag="esp")
                    for tl in range(tn):
                        t = t0 + tl
                        b = int(block_of_tile[t])
                        # edp = a_dst[dst] per edge
                        nc.tensor.matmul(esp[:, tl, :], ST_sb[:NPB, t, :],
                                         adst1s[:NPB, b, :], start=True, stop=True)
                    # chunk-batched a_src add + leaky-relu + exp + exp*h
                    esl = sb1.tile([128, CHUNK, H1], F32, tag="esl", bufs=3)
                    nc.vector.tensor_tensor(esl[:, 0:tn, :], esp[:, 0:tn, :],
                                            g[:, 0:tn, 0:H1], OP.add)
                    eslp = sb1.tile([128, CHUNK, H1], F32, tag="eslp", bufs=3)
                    nc.scalar.activation(eslp[:, 0:tn, :], esl[:, 0:tn, :],
                                         AF.Prelu, alpha=NEG_SLOPE)
                    expf = sb1.tile([128, CHUNK, H1], F32, tag="expf", bufs=3)
                    nc.scalar.activation(expf[:, 0:tn, :], eslp[:, 0:tn, :],
                                         AF.Exp)
                    # exhs = [exp(5) | zero pad(3) | exp*h(320)], 16B-aligned
                    exhs = sb1.tile([128, CHUNK, 8 + F1], BF16, tag="exhs")
                    nc.vector.tensor_copy(exhs[:, 0:tn, 0:H1], expf[:, 0:tn, :])
                    nc.vector.memset(exhs[:, 0:tn, H1:8], 0.0)
                    g4 = g[:, 0:tn, 2 * H1:2 * H1 + 3 * C1].rearrange(
                        "p t (h c) -> p t h c", h=3)
                    ex3 = exhs[:, 0:tn, 0:3].unsqueeze(3).broadcast_to(
                        (128, tn, 3, C1))
                    o4 = exhs[:, 0:tn, 8:8 + 3 * C1].rearrange(
                        "p t (h c) -> p t h c", h=3)
                    nc.vector.tensor_tensor(o4, g4, ex3, OP.mult)
                    for tl in range(tn):
                        for h in (3, 4):
                            nc.scalar.activation(
                                exhs[:, tl, 8 + C1 * h:8 + C1 * (h + 1)],
                                g[:, tl, 2 * H1 + C1 * h:2 * H1 + C1 * (h + 1)],
                                AF.Identity, scale=expf[:, tl, h:h + 1])
                    for tl in range(tn):
                        t = t0 + tl
                        b = int(block_of_tile[t])
                        first = t == tile_ofs[b]
                        last = t == tile_ofs[b + 1] - 1
                        if first:
                            usp = ps_u.tile([128, 8 + F1], F32, tag="usp")
                        nc.tensor.matmul(usp[:, :], S_sb[:, t, :],
                                         exhs[:, tl, :], start=first, stop=last)
                        if last:
                            # epilogue: alpha normalize + bias + relu
                            rec = epi.tile([128, H1], F32, tag="rec")
                            nc.vector.tensor_scalar_add(rec[:NPB, :],
                                                        usp[:NPB, 0:H1], EPS)
                            nc.vector.reciprocal(rec[:NPB, :], rec[:NPB, :])
                            o1 = epi.tile([128, F1], F32, tag="o1")
                            u4 = usp[:NPB, 8:].rearrange("p (h c) -> p h c", h=H1)
                            r4 = rec[:NPB, :].unsqueeze(2).broadcast_to(
                                (NPB, H1, C1))
                            o14 = o1[:NPB, :].rearrange("p (h c) -> p h c", h=H1)
                            nc.vector.tensor_tensor(o14, u4, r4, OP.mult)
                            nc.vector.tensor_tensor(o1[:NPB, :], o1[:NPB, :],
                                                    b1s[:NPB, :], OP.add)
                            o1r = epi.tile([128, F1], BF16, tag="o1r")
                            nc.scalar.activation(o1r[:NPB, :], o1[:NPB, :],
                                                 AF.Relu)
                            # sink: h2 = relu(out1) @ [W2|W2a_src2|W2a_dst2]
                            h1T = epi.tile([128, 3, NPB], BF16, tag="h1T")
                            for k in range(3):
                                w3 = min(128, F1 - 128 * k)
                                tp = ps_t.tile([128, NPB], BF16, tag="tp")
                                nc.tensor.transpose(
                                    tp[:w3, :], o1r[:NPB, 128 * k:128 * k + w3],
                                    idents[:NPB, :NPB])
                                nc.vector.tensor_copy(h1T[:w3, k, :], tp[:w3, :])
                            h2ps = ps_t.tile([128, F2 + 2], F32, tag="h2ps")
                            for k in range(3):
                                w3 = min(128, F1 - 128 * k)
                                nc.tensor.matmul(h2ps[:NPB, :], h1T[:w3, k, :],
                                                 W2cs[:w3, k, :],
                                                 start=(k == 0), stop=(k == 2))
                            agrow = epi.tile([128, F2 + 2], BF16, tag="agrow")
                            nc.vector.tensor_copy(agrow[:NPB, :],
                                                  h2ps[:NPB, :])
                            nc.sync.dma_start(ag_in[NPB * b:NPB * (b + 1), :],
                                              agrow[:NPB, :])
                            nc.vector.tensor_copy(adst2s[:NPB, b, :],
                                                  h2ps[:NPB, F2 + 1:F2 + 2])

            nc.gpsimd.collective_compute(
                "AllGather", mybir.AluOpType.bypass,
                replica_groups=[list(range(N_CORES))],
                ins=[ag_in.opt()], outs=[h2pack.opt()])
            nc.sync.dma_start(h2tab[0:N_NODES // 2, 0:F2 + 2],
                              h2pack[0:N_NODES // 2, :])
            nc.scalar.dma_start(h2tab[N_NODES // 2:, 0:F2 + 2],
                                h2pack[N_NODES // 2:, :])

            # ---- layer 2 edge sweep ---------------------------------------
            with (
                tc.tile_pool(name="gbuf2", bufs=12) as gbuf2,
                tc.tile_pool(name="sb2", bufs=4) as sb2,
                tc.tile_pool(name="ps_e2", bufs=1, space="PSUM") as ps_e2,
                tc.tile_pool(name="ps_u2", bufs=2, space="PSUM") as ps_u2,
                tc.tile_pool(name="epi2", bufs=2) as epi2,
            ):
                # a_dst2 expanded to edge slots per chunk, one PSUM bank
                ep2 = ps_e2.tile([128, Ttot, 1], F32)
                usp2 = None
                for ch in range(n_chunks):
                    t0 = ch * CHUNK
                    tn = min(CHUNK, Ttot - t0)
                    for tl in range(tn):
                        t = t0 + tl
                        b = int(block_of_tile[t])
                        nc.tensor.matmul(ep2[:, t, :], ST_sb[:NPB, t, :],
                                         adst2s[:NPB, b, :], start=True, stop=True)
                    g2 = gbuf2.tile([128, CHUNK, ROW2], BF16, tag="g2")
                    nc.gpsimd.dma_gather(
                        g2[:, 0:tn, :], h2tab[:],
                        src16b[:, t0 * 8:t0 * 8 + tn * 8],
                        num_idxs=tn * TILE_E, num_idxs_reg=tn * TILE_E,
                        elem_size=ROW2, queue_num=ch % 4)
                    es2 = sb2.tile([128, CHUNK, 1], F32, tag="es2")
                    nc.vector.tensor_tensor(es2[:, 0:tn, :],
                                            g2[:, 0:tn, F2:F2 + 1],
                                            ep2[:, t0:t0 + tn, :], OP.add)
                    es2l = sb2.tile([128, CHUNK, 1], F32, tag="es2l")
                    nc.scalar.activation(es2l[:, 0:tn, :], es2[:, 0:tn, :],
                                         AF.Prelu, alpha=NEG_SLOPE)
                    exhs2 = sb2.tile([128, CHUNK, 1 + F2], BF16, tag="exhs2")
                    nc.scalar.activation(exhs2[:, 0:tn, 0:1], es2l[:, 0:tn, :],
                                         AF.Exp)
                    ex2 = exhs2[:, 0:tn, 0:1].broadcast_to((128, tn, F2))
                    nc.vector.tensor_tensor(exhs2[:, 0:tn, 1:], g2[:, 0:tn, 0:F2],
                                            ex2, OP.mult)
                    for tl in range(tn):
                        t = t0 + tl
                        b = int(block_of_tile[t])
                        first = t == tile_ofs[b]
                        last = t == tile_ofs[b + 1] - 1
                        if first:
                            usp2 = ps_u2.tile([128, 1 + F2], F32, tag="usp2")
                        nc.tensor.matmul(usp2[:, :], S_sb[:, t, :],
                                         exhs2[:, tl, :], start=first, stop=last)
                        if last:
                            rec2 = epi2.tile([128, 1], F32, tag="rec2")
                            nc.vector.tensor_scalar_add(rec2[:NPB, :],
                                                        usp2[:NPB, 0:1], EPS)
                            nc.vector.reciprocal(rec2[:NPB, :], rec2[:NPB, :])
                            o2 = epi2.tile([128, F2], F32, tag="o2")
                            nc.vector.scalar_tensor_tensor(
                                o2[:NPB, :], usp2[:NPB, 1:], rec2[:NPB, :],
                                b2s[:NPB, :], OP.mult, OP.add)
                            o2r = epi2.tile([128, F2], F32, tag="o2r")
                            nc.scalar.activation(o2r[:NPB, :], o2[:NPB, :],
                                                 AF.Relu)
                            nc.sync.dma_start(out_d[NPB * b:NPB * (b + 1), :],
                                              o2r[:NPB, :])

    nc.compile()
    return nc


def kernel(**inputs) -> np.ndarray:
    import time

    from concourse.bass_utils import run_bass_kernel_spmd

    shared, per_core, tile_ofs, Ttot, Epad = _host_prep(inputs)
    nc = _build_program(tile_ofs, Ttot, Epad)

    in_maps = []
    for c in range(N_CORES):
        m = dict(shared)
        m.update(per_core[c])
        in_maps.append(m)
    res = None
    for attempt in range(3):
        try:
            res = run_bass_kernel_spmd(nc, in_maps, list(range(N_CORES)))
            break
        except Exception:
            if attempt == 2:
                raise
            time.sleep(5)
    out = np.concatenate([res.results[c]["out"] for c in range(N_CORES)], axis=0)
    return np.ascontiguousarray(out.astype(np.float32))


# revision 26
# speedup vs baseline: 1.0493x; 1.0493x over previous
"""Trainium2 Bass kernel for the 2-layer GAT block (nn_GATblock_58282706206740).

Strategy (8 NeuronCores, SPMD, dst-sharded):
  - Edges (incl. self-loops) sharded by destination-node range: core c owns
    dst nodes [1250c, 1250(c+1)), split into 10 blocks of 125. Per-(core,
    block) edge lists padded to a common per-block tile count (max over
    cores) of 128-edge tiles, so one program serves all cores.
  - Phase A (replicated, bf16): node table rows [asrc1(5)|adst1(5)|h1(320)]
    written to DRAM (768B rows); per-edge rows arrive via gpsimd dma_gather.
  - dma_gather descriptor generation is the machine's serial bottleneck
    (~8.4 ns/idx on one Q7 core pair); gathers round-robin over 4 SWDGE
    queues so 4 desc-gens+transfers run concurrently (~4x).
  - Scores e = a_src[src]+a_dst[dst] are summed in PSUM by two accumulating
    PE matmuls per tile (a_dst expands via the fp8 ST one-hot; a_src copies
    in via an identity matmul). leaky-relu/exp run chunk-batched on the
    Scalar engine (Prelu/Exp share one activation table); exp*h is one
    chunk-batched DVE multiply in bf16.
  - Segment softmax-sum + message aggregation is one fused PE matmul per
    tile with rhs [exp | exp*h] (bf16) against the fp8 S one-hot (0/1 is
    exact in fp8; fp8 lhsT pairs with bf16 rhs, verified exact on HW).
    Softmax skips the max subtraction (scores provably small) and keeps
    the reference's +1e-16 denominator epsilon.
  - Between layers one AllGather exchanges the bf16 [h2 | a_src2] table.
"""
import sys

sys.path.insert(0, "/opt/trn_rl_repo")

import ml_dtypes
import numpy as np

N_NODES = 10000
N_CORES = 8
NPC = N_NODES // N_CORES          # 1250
B_BLOCKS = 10
NPB = NPC // B_BLOCKS             # 125
TILE_E = 128
CHUNK = 8                         # tiles per gather call (1024 idx)
EPS = 1e-16
NEG_SLOPE = 0.2
F0, F1, F2, H1, C1 = 128, 320, 64, 5, 64
ROW1 = 384                        # [asrc1(5) | adst1(5) | h1(320) | pad]
ROW2 = 128                        # [h2(64) | asrc2(1) | pad]


def _build_partition(edge_index):
    src = np.concatenate([edge_index[0].astype(np.int64),
                          np.arange(N_NODES, dtype=np.int64)])
    dst = np.concatenate([edge_index[1].astype(np.int64),
                          np.arange(N_NODES, dtype=np.int64)])
    core = dst // NPC
    block = (dst % NPC) // NPB
    col = dst % NPB

    cnt = np.zeros((N_CORES, B_BLOCKS), dtype=np.int64)
    np.add.at(cnt, (core, block), 1)
    T_b = np.ceil(cnt.max(axis=0) / TILE_E).astype(np.int64)
    tile_ofs = np.concatenate([[0], np.cumsum(T_b)])
    Ttot = int(tile_ofs[-1])
    Epad = Ttot * TILE_E

    src_sl = np.zeros((N_CORES, Epad), dtype=np.int64)
    col_sl = np.full((N_CORES, Epad), 200.0, dtype=np.float32)
    order = np.lexsort((dst, core * B_BLOCKS + block))
    s_src, s_core, s_block, s_col = src[order], core[order], block[order], col[order]
    idx = 0
    for c in range(N_CORES):
        for b in range(B_BLOCKS):
            n = int(cnt[c, b])
            base = int(tile_ofs[b]) * TILE_E
            sl = slice(idx, idx + n)
            assert np.all(s_core[sl] == c) and np.all(s_block[sl] == b)
            src_sl[c, base:base + n] = s_src[sl]
            col_sl[c, base:base + n] = s_col[sl]
            idx += n
    assert idx == len(src)
    return src_sl, col_sl, tile_ofs, Ttot, Epad


def _wrap_idx16(idx):
    a = idx.astype(np.int16).reshape(-1, 16).T
    return np.tile(a, (8, 1))


def _host_prep(inputs):
    x = np.asarray(inputs["x"], dtype=np.float32)
    W1 = np.asarray(inputs["W1"], dtype=np.float32)
    att_src1 = np.asarray(inputs["att_src1"], dtype=np.float32)
    att_dst1 = np.asarray(inputs["att_dst1"], dtype=np.float32)
    b1 = np.asarray(inputs["b1"], dtype=np.float32)
    W2 = np.asarray(inputs["W2"], dtype=np.float32)
    att_src2 = np.asarray(inputs["att_src2"], dtype=np.float32)
    att_dst2 = np.asarray(inputs["att_dst2"], dtype=np.float32)
    b2 = np.asarray(inputs["b2"], dtype=np.float32)
    ei = np.asarray(inputs["edge_index"])

    src_sl, col_sl, tile_ofs, Ttot, Epad = _build_partition(ei)

    bf16 = ml_dtypes.bfloat16
    fp8 = ml_dtypes.float8_e4m3fn

    # W1cat = [W1@Asrc(5) | W1@Adst(5) | W1(320)] -> node row = x @ W1cat
    W1Asrc = np.stack([W1[:, 64 * h:64 * h + 64] @ att_src1[h] for h in range(H1)], axis=1)
    W1Adst = np.stack([W1[:, 64 * h:64 * h + 64] @ att_dst1[h] for h in range(H1)], axis=1)
    W1cat = np.concatenate([W1Asrc, W1Adst, W1], axis=1).astype(bf16)  # [128, 330]

    # W2cat tiles: [128, 3, 66] = [W2 | W2@asrc2 | W2@adst2] zero-padded
    W2c = np.concatenate([W2, (W2 @ att_src2[0])[:, None],
                          (W2 @ att_dst2[0])[:, None]], axis=1)  # [320, 66]
    W2p = np.zeros((384, 66), dtype=np.float32)
    W2p[:320] = W2c
    W2cat = np.ascontiguousarray(W2p.reshape(3, 128, 66).transpose(1, 0, 2)).astype(bf16)

    xT = np.ascontiguousarray(x.T)
    shared = dict(
        xT=xT.astype(bf16),                              # [128, 10000]
        W1cat=W1cat,
        W2cat=W2cat,
        ident=np.eye(128, dtype=np.float32).astype(bf16),
        b1rep=np.broadcast_to(b1, (128, F1)).copy(),
        b2rep=np.broadcast_to(b2, (128, F2)).copy(),
    )
    # h1 table row permutation: phase A writes groups of 4 node tiles as
    # [p, i] -> row 512g + 4p + i so each partition's 4 rows are contiguous
    # (3KB runs -> 4x fewer DMA descriptors). Tail nodes keep identity rows.
    n_full = (N_NODES // 512) * 512
    nn = np.arange(N_NODES, dtype=np.int64)
    perm1 = np.where(
        nn < n_full,
        (nn // 512) * 512 + (nn % 128) * 4 + (nn % 512) // 128,
        nn)

    d = np.arange(128, dtype=np.float32)
    per_core = []
    for c in range(N_CORES):
        colf = np.ascontiguousarray(col_sl[c].reshape(Ttot, TILE_E).T)  # [128, Ttot]
        S = (colf[:, :, None] == d[None, None, :])                      # [e,t,d]
        per_core.append(dict(
            src16=_wrap_idx16(perm1[src_sl[c]]),
            src16b=_wrap_idx16(src_sl[c]),
            Sb=np.ascontiguousarray(S).astype(fp8),
            STb=np.ascontiguousarray(np.transpose(S, (2, 1, 0))).astype(fp8),
            xTc=np.ascontiguousarray(xT[:, c * NPC:(c + 1) * NPC]).astype(bf16),
        ))
    return shared, per_core, tile_ofs, Ttot, Epad


def _build_program(tile_ofs, Ttot, Epad):
    import concourse.bacc as bacc
    import concourse.mybir as mybir
    from concourse import tile

    dt = mybir.dt
    F32 = dt.float32
    BF16 = dt.bfloat16
    FP8 = dt.float8e4
    AF = mybir.ActivationFunctionType
    OP = mybir.AluOpType

    B = B_BLOCKS
    tile_ofs = [int(v) for v in tile_ofs]
    block_of_tile = np.zeros(Ttot, dtype=np.int64)
    for b in range(B):
        block_of_tile[tile_ofs[b]:tile_ofs[b + 1]] = b
    n_chunks = (Ttot + CHUNK - 1) // CHUNK
    n_node_tiles = (N_NODES + 127) // 128

    nc = bacc.Bacc("TRN2", target_bir_lowering=False, debug=False,
                   num_devices=N_CORES, num_swdge_queues=4)

    xT_d = nc.dram_tensor("xT", [F0, N_NODES], BF16, kind="ExternalInput")
    W1c_d = nc.dram_tensor("W1cat", [F0, 2 * H1 + F1], BF16, kind="ExternalInput")
    W2c_d = nc.dram_tensor("W2cat", [128, 3, F2 + 2], BF16, kind="ExternalInput")
    ident_d = nc.dram_tensor("ident", [128, 128], BF16, kind="ExternalInput")
    b1_d = nc.dram_tensor("b1rep", [128, F1], F32, kind="ExternalInput")
    b2_d = nc.dram_tensor("b2rep", [128, F2], F32, kind="ExternalInput")
    src16_d = nc.dram_tensor("src16", [128, Epad // 16], dt.int16, kind="ExternalInput")
    src16b_d = nc.dram_tensor("src16b", [128, Epad // 16], dt.int16, kind="ExternalInput")
    S_d = nc.dram_tensor("Sb", [128, Ttot, 128], FP8, kind="ExternalInput")
    ST_d = nc.dram_tensor("STb", [128, Ttot, 128], FP8, kind="ExternalInput")
    xTc_d = nc.dram_tensor("xTc", [F0, NPC], BF16, kind="ExternalInput")
    out_d = nc.dram_tensor("out", [NPC, F2], F32, kind="ExternalOutput")

    with tile.TileContext(nc) as tc:
        with (
            tc.tile_pool(name="dram", bufs=1, space="DRAM") as dram,
            tc.tile_pool(name="const", bufs=1) as cpool,
        ):
            h1tab = dram.tile([N_NODES, ROW1], BF16)
            ag_in = dram.tile([NPC, F2 + 2], BF16)
            h2pack = dram.tile([N_NODES, F2 + 2], BF16, addr_space="Shared")
            h2tab = dram.tile([N_NODES, ROW2], BF16)

            W1cs = cpool.tile([F0, 2 * H1 + F1], BF16)
            nc.sync.dma_start(W1cs[:], W1c_d[:])
            W2cs = cpool.tile([128, 3, F2 + 2], BF16)
            nc.sync.dma_start(W2cs[:], W2c_d[:])
            idents = cpool.tile([128, 128], BF16)
            nc.sync.dma_start(idents[:], ident_d[:])
            b1s = cpool.tile([128, F1], F32)
            nc.sync.dma_start(b1s[:], b1_d[:])
            b2s = cpool.tile([128, F2], F32)
            nc.sync.dma_start(b2s[:], b2_d[:])
            src16 = cpool.tile([128, Epad // 16], dt.int16)
            nc.sync.dma_start(src16[:], src16_d[:])
            src16b = cpool.tile([128, Epad // 16], dt.int16)
            nc.sync.dma_start(src16b[:], src16b_d[:])
            S_sb = cpool.tile([128, Ttot, 128], FP8)
            nc.sync.dma_start(S_sb[:], S_d[:])
            ST_sb = cpool.tile([128, Ttot, 128], FP8)
            nc.sync.dma_start(ST_sb[:], ST_d[:])
            xTcs = cpool.tile([F0, NPC], BF16)
            nc.sync.dma_start(xTcs[:], xTc_d[:])
            adst1s = cpool.tile([128, B, H1], BF16)
            adst2s = cpool.tile([128, B, 1], BF16)

            # ---- phase A: node table + own-dst adst1 -----------------------
            with (
                tc.tile_pool(name="pAx", bufs=1) as pAx,
                tc.tile_pool(name="pA", bufs=3) as pA,
                tc.tile_pool(name="psA", bufs=3, space="PSUM") as psA,
            ):
                for b in range(B):
                    pa = psA.tile([128, H1], F32, tag="pa")
                    nc.tensor.matmul(pa[:NPB, :], xTcs[:, NPB * b:NPB * (b + 1)],
                                     W1cs[:, H1:2 * H1], start=True, stop=True)
                    nc.vector.tensor_copy(adst1s[:NPB, b, :], pa[:NPB, :])
                xTs = pAx.tile([F0, N_NODES], BF16)
                XCH = 2560
                for xo in range(0, N_NODES, XCH):
                    xw = min(XCH, N_NODES - xo)
                    nc.sync.dma_start(xTs[:, xo:xo + xw], xT_d[:, xo:xo + xw])
                GRP = 4
                for nt0 in range(0, n_node_tiles, GRP):
                    gn = min(GRP, n_node_tiles - nt0)
                    row = pA.tile([128, GRP, ROW1], BF16, tag="row")
                    for i in range(gn):
                        nt = nt0 + i
                        w = min(128, N_NODES - 128 * nt)
                        ph = psA.tile([128, 2 * H1 + F1], F32, tag="ph")
                        nc.tensor.matmul(ph[:w, :], xTs[:, 128 * nt:128 * nt + w],
                                         W1cs[:], start=True, stop=True)
                        if i % 2 == 0:
                            nc.vector.tensor_copy(row[:w, i, 0:2 * H1 + F1],
                                                  ph[:w, :])
                        else:
                            nc.scalar.activation(row[:w, i, 0:2 * H1 + F1],
                                                 ph[:w, :], AF.Copy)
                    if 128 * (nt0 + gn) <= n_node_tiles * 128 and gn == GRP:
                        # permuted rows: row 512g + 4p + i; [p, i] contiguous
                        dst = h1tab[512 * (nt0 // GRP):512 * (nt0 // GRP + 1), :]
                        dst = dst.rearrange("(p i) r -> p i r", i=GRP)
                        nc.sync.dma_start(dst, row[:])
                    else:
                        for i in range(gn):
                            nt = nt0 + i
                            w = min(128, N_NODES - 128 * nt)
                            nc.sync.dma_start(
                                h1tab[128 * nt:128 * nt + w, 0:2 * H1 + F1],
                                row[:w, i, 0:2 * H1 + F1])

            # ---- layer 1 edge sweep ---------------------------------------
            with (
                tc.tile_pool(name="gbuf", bufs=12) as gbuf,
                tc.tile_pool(name="sb1", bufs=4) as sb1,
                tc.tile_pool(name="ps_es", bufs=3, space="PSUM") as ps_es,
                tc.tile_pool(name="ps_u", bufs=2, space="PSUM") as ps_u,
                tc.tile_pool(name="ps_t", bufs=1, space="PSUM") as ps_t,
                tc.tile_pool(name="epi", bufs=2) as epi,
            ):
                usp = None
                for ch in range(n_chunks):
                    t0 = ch * CHUNK
                    tn = min(CHUNK, Ttot - t0)
                    g = gbuf.tile([128, CHUNK, ROW1], BF16, tag="g")
                    nc.gpsimd.dma_gather(
                        g[:, 0:tn, :], h1tab[:],
                        src16[:, t0 * 8:t0 * 8 + tn * 8],
                        num_idxs=tn * TILE_E, num_idxs_reg=tn * TILE_E,
                        elem_size=ROW1, queue_num=ch % 4)
                    esp = ps_es.tile([128, CHUNK, H1], F32, tag="esp")
                    for tl in range(tn):
                        t = t0 + tl
                        b = int(block_of_tile[t])
                        # edp = a_dst[dst] per edge
                        nc.tensor.matmul(esp[:, tl, :], ST_sb[:NPB, t, :],
                                         adst1s[:NPB, b, :], start=True, stop=True)
                    # chunk-batched a_src add + leaky-relu + exp + exp*h
                    esl = sb1.tile([128, CHUNK, H1], F32, tag="esl", bufs=3)
                    nc.vector.tensor_tensor(esl[:, 0:tn, :], esp[:, 0:tn, :],
                                            g[:, 0:tn, 0:H1], OP.add)
                    eslp = sb1.tile([128, CHUNK, H1], F32, tag="eslp", bufs=3)
                    nc.scalar.activation(eslp[:, 0:tn, :], esl[:, 0:tn, :],
                                         AF.Prelu, alpha=NEG_SLOPE)
                    expf = sb1.tile([128, CHUNK, H1], F32, tag="expf", bufs=3)
                    nc.scalar.activation(expf[:, 0:tn, :], eslp[:, 0:tn, :],
                                         AF.Exp)
                    # exhs = [exp(5) | zero pad(3) | exp*h(320)], 16B-aligned
                    exhs = sb1.tile([128, CHUNK, 8 + F1], BF16, tag="exhs")
                    nc.vector.tensor_copy(exhs[:, 0:tn, 0:H1], expf[:, 0:tn, :])
                    nc.vector.memset(exhs[:, 0:tn, H1:8], 0.0)
                    g4 = g[:, 0:tn, 2 * H1:2 * H1 + F1].rearrange(
                        "p t (h c) -> p t h c", h=H1)
                    ex4 = exhs[:, 0:tn, 0:H1].unsqueeze(3).broadcast_to(
                        (128, tn, H1, C1))
                    o4 = exhs[:, 0:tn, 8:].rearrange("p t (h c) -> p t h c", h=H1)
                    nc.vector.tensor_tensor(o4, g4, ex4, OP.mult)
                    for tl in range(tn):
                        t = t0 + tl
                        b = int(block_of_tile[t])
                        first = t == tile_ofs[b]
                        last = t == tile_ofs[b + 1] - 1
                        if first:
                            usp = ps_u.tile([128, 8 + F1], F32, tag="usp")
                        nc.tensor.matmul(usp[:, :], S_sb[:, t, :],
                                         exhs[:, tl, :], start=first, stop=last)
                        if last:
                            # epilogue: alpha normalize + bias + relu
                            rec = epi.tile([128, H1], F32, tag="rec")
                            nc.vector.tensor_scalar_add(rec[:NPB, :],
                                                        usp[:NPB, 0:H1], EPS)
                            nc.vector.reciprocal(rec[:NPB, :], rec[:NPB, :])
                            o1 = epi.tile([128, F1], F32, tag="o1")
                            u4 = usp[:NPB, 8:].rearrange("p (h c) -> p h c", h=H1)
                            r4 = rec[:NPB, :].unsqueeze(2).broadcast_to(
                                (NPB, H1, C1))
                            o14 = o1[:NPB, :].rearrange("p (h c) -> p h c", h=H1)
                            nc.vector.tensor_tensor(o14, u4, r4, OP.mult)
                            nc.vector.tensor_tensor(o1[:NPB, :], o1[:NPB, :],
                                                    b1s[:NPB, :], OP.add)
                            o1r = epi.tile([128, F1], BF16, tag="o1r")
                            nc.scalar.activation(o1r[:NPB, :], o1[:NPB, :],
                                                 AF.Relu)
                            # sink: h2 = relu(out1) @ [W2|W2a_src2|W2a_dst2]
                            h1T = epi.tile([128, 3, NPB], BF16, tag="h1T")
                            for k in range(3):
                                w3 = min(128, F1 - 128 * k)
                                tp = ps_t.tile([128, NPB], BF16, tag="tp")
                                nc.tensor.transpose(
                                    tp[:w3, :], o1r[:NPB, 128 * k:128 * k + w3],
                                    idents[:NPB, :NPB])
                                nc.vector.tensor_copy(h1T[:w3, k, :], tp[:w3, :])
                            h2ps = ps_t.tile([128, F2 + 2], F32, tag="h2ps")
                            for k in range(3):
                                w3 = min(128, F1 - 128 * k)
                                nc.tensor.matmul(h2ps[:NPB, :], h1T[:w3, k, :],
                                                 W2cs[:w3, k, :],
                                                 start=(k == 0), stop=(k == 2))
                            agrow = epi.tile([128, F2 + 2], BF16, tag="agrow")
                            nc.vector.tensor_copy(agrow[:NPB, :],
                                                  h2ps[:NPB, :])
                            nc.sync.dma_start(ag_in[NPB * b:NPB * (b + 1), :],
                                              agrow[:NPB, :])
                            nc.vector.tensor_copy(adst2s[:NPB, b, :],
                                                  h2ps[:NPB, F2 + 1:F2 + 2])

            nc.gpsimd.collective_compute(
                "AllGather", mybir.AluOpType.bypass,
                replica_groups=[list(range(N_CORES))],
                ins=[ag_in.opt()], outs=[h2pack.opt()])
            nc.sync.dma_start(h2tab[0:N_NODES // 2, 0:F2 + 2],
                              h2pack[0:N_NODES // 2, :])
            nc.scalar.dma_start(h2tab[N_NODES // 2:, 0:F2 + 2],
                                h2pack[N_NODES // 2:, :])

            # ---- layer 2 edge sweep ---------------------------------------
            with (
                tc.tile_pool(name="gbuf2", bufs=12) as gbuf2,
                tc.tile_pool(name="sb2", bufs=4) as sb2,
                tc.tile_pool(name="ps_e2", bufs=1, space="PSUM") as ps_e2,
                tc.tile_pool(name="ps_u2", bufs=2, space="PSUM") as ps_u2,
                tc.tile_pool(name="epi2", bufs=2) as epi2,
            ):
                # a_dst2 expanded to edge slots per chunk, one PSUM bank
                ep2 = ps_e2.tile([128, Ttot, 1], F32)
                usp2 = None
                for ch in range(n_chunks):
                    t0 = ch * CHUNK
                    tn = min(CHUNK, Ttot - t0)
                    for tl in range(tn):
                        t = t0 + tl
                        b = int(block_of_tile[t])
                        nc.tensor.matmul(ep2[:, t, :], ST_sb[:NPB, t, :],
                                         adst2s[:NPB, b, :], start=True, stop=True)
                    g2 = gbuf2.tile([128, CHUNK, ROW2], BF16, tag="g2")
                    nc.gpsimd.dma_gather(
                        g2[:, 0:tn, :], h2tab[:],
                        src16b[:, t0 * 8:t0 * 8 + tn * 8],
                        num_idxs=tn * TILE_E, num_idxs_reg=tn * TILE_E,
                        elem_size=ROW2, queue_num=ch % 4)
                    es2 = sb2.tile([128, CHUNK, 1], F32, tag="es2")
                    nc.vector.tensor_tensor(es2[:, 0:tn, :],
                                            g2[:, 0:tn, F2:F2 + 1],
                                            ep2[:, t0:t0 + tn, :], OP.add)
                    es2l = sb2.tile([128, CHUNK, 1], F32, tag="es2l")
                    nc.scalar.activation(es2l[:, 0:tn, :], es2[:, 0:tn, :],
                                         AF.Prelu, alpha=NEG_SLOPE)
                    exhs2 = sb2.tile([128, CHUNK, 1 + F2], BF16, tag="exhs2")
                    nc.scalar.activation(exhs2[:, 0:tn, 0:1], es2l[:, 0:tn, :],
                                         AF.Exp)
                    ex2 = exhs2[:, 0:tn, 0:1].broadcast_to((128, tn, F2))
                    nc.vector.tensor_tensor(exhs2[:, 0:tn, 1:], g2[:, 0:tn, 0:F2],
                                            ex2, OP.mult)
                    for tl in range(tn):
                        t = t0 + tl
                        b = int(block_of_tile[t])
                        first = t == tile_ofs[b]
                        last = t == tile_ofs[b + 1] - 1
                        if first:
                            usp2 = ps_u2.tile([128, 1 + F2], F32, tag="usp2")
                        nc.tensor.matmul(usp2[:, :], S_sb[:, t, :],
                                         exhs2[:, tl, :], start=first, stop=last)
                        if last:
                            rec2 = epi2.tile([128, 1], F32, tag="rec2")
                            nc.vector.tensor_scalar_add(rec2[:NPB, :],
                                                        usp2[:NPB, 0:1], EPS)
                            nc.vector.reciprocal(rec2[:NPB, :], rec2[:NPB, :])
                            o2 = epi2.tile([128, F2], F32, tag="o2")
                            nc.vector.scalar_tensor_tensor(
                                o2[:NPB, :], usp2[:NPB, 1:], rec2[:NPB, :],
                                b2s[:NPB, :], OP.mult, OP.add)
                            o2r = epi2.tile([128, F2], F32, tag="o2r")
                            nc.scalar.activation(o2r[:NPB, :], o2[:NPB, :],
                                                 AF.Relu)
                            nc.sync.dma_start(out_d[NPB * b:NPB * (b + 1), :],
                                              o2r[:NPB, :])

    nc.compile()
    return nc


def kernel(**inputs) -> np.ndarray:
    import time

    from concourse.bass_utils import run_bass_kernel_spmd

    shared, per_core, tile_ofs, Ttot, Epad = _host_prep(inputs)
    nc = _build_program(tile_ofs, Ttot, Epad)

    in_maps = []
    for c in range(N_CORES):
        m = dict(shared)
        m.update(per_core[c])
        in_maps.append(m)
    res = None
    for attempt in range(3):
        try:
            res = run_bass_kernel_spmd(nc, in_maps, list(range(N_CORES)))
            break
        except Exception:
            if attempt == 2:
                raise
            time.sleep(5)
    out = np.concatenate([res.results[c]["out"] for c in range(N_CORES)], axis=0)
    return np.ascontiguousarray(out.astype(np.float32))


# revision 27
# speedup vs baseline: 1.4476x; 1.3796x over previous
"""Trainium2 Bass kernel for the 2-layer GAT block (nn_GATblock_58282706206740).

Strategy (8 NeuronCores, SPMD, dst-sharded):
  - Edges (incl. self-loops) sharded by destination-node range: core c owns
    dst nodes [1250c, 1250(c+1)), split into 10 blocks of 125. Per-(core,
    block) edge lists padded to a common per-block tile count (max over
    cores) of 128-edge tiles, so one program serves all cores.
  - Phase A (replicated, bf16): node table rows [asrc1(5)|adst1(5)|h1(320)]
    written to DRAM (768B rows); per-edge rows arrive via gpsimd dma_gather.
  - dma_gather descriptor generation is the machine's serial bottleneck
    (~8.4 ns/idx on one Q7 core pair); gathers round-robin over 4 SWDGE
    queues so 4 desc-gens+transfers run concurrently (~4x).
  - Scores e = a_src[src]+a_dst[dst] are summed in PSUM by two accumulating
    PE matmuls per tile (a_dst expands via the fp8 ST one-hot; a_src copies
    in via an identity matmul). leaky-relu/exp run chunk-batched on the
    Scalar engine (Prelu/Exp share one activation table); exp*h is one
    chunk-batched DVE multiply in bf16.
  - Segment softmax-sum + message aggregation is one fused PE matmul per
    tile with rhs [exp | exp*h] (bf16) against the fp8 S one-hot (0/1 is
    exact in fp8; fp8 lhsT pairs with bf16 rhs, verified exact on HW).
    Softmax skips the max subtraction (scores provably small) and keeps
    the reference's +1e-16 denominator epsilon.
  - Between layers one AllGather exchanges the bf16 [h2 | a_src2] table.
"""
import sys

sys.path.insert(0, "/opt/trn_rl_repo")

import ml_dtypes
import numpy as np

N_NODES = 10000
N_CORES = 8
NPC = N_NODES // N_CORES          # 1250
B_BLOCKS = 10
NPB = NPC // B_BLOCKS             # 125
TILE_E = 128
CHUNK = 8                         # tiles per gather call (1024 idx)
EPS = 1e-16
NEG_SLOPE = 0.2
F0, F1, F2, H1, C1 = 128, 320, 64, 5, 64
ROW1 = 384                        # [asrc1(5) | adst1(5) | h1(320) | pad]
ROW2 = 128                        # [h2(64) | asrc2(1) | pad]


def _build_partition(edge_index):
    src = np.concatenate([edge_index[0].astype(np.int64),
                          np.arange(N_NODES, dtype=np.int64)])
    dst = np.concatenate([edge_index[1].astype(np.int64),
                          np.arange(N_NODES, dtype=np.int64)])
    core = dst // NPC
    block = (dst % NPC) // NPB
    col = dst % NPB

    cnt = np.zeros((N_CORES, B_BLOCKS), dtype=np.int64)
    np.add.at(cnt, (core, block), 1)
    T_b = np.ceil(cnt.max(axis=0) / TILE_E).astype(np.int64)
    tile_ofs = np.concatenate([[0], np.cumsum(T_b)])
    Ttot = int(tile_ofs[-1])
    Epad = Ttot * TILE_E

    src_sl = np.zeros((N_CORES, Epad), dtype=np.int64)
    col_sl = np.full((N_CORES, Epad), 200.0, dtype=np.float32)
    order = np.lexsort((dst, core * B_BLOCKS + block))
    s_src, s_core, s_block, s_col = src[order], core[order], block[order], col[order]
    idx = 0
    for c in range(N_CORES):
        for b in range(B_BLOCKS):
            n = int(cnt[c, b])
            base = int(tile_ofs[b]) * TILE_E
            sl = slice(idx, idx + n)
            assert np.all(s_core[sl] == c) and np.all(s_block[sl] == b)
            src_sl[c, base:base + n] = s_src[sl]
            col_sl[c, base:base + n] = s_col[sl]
            idx += n
    assert idx == len(src)
    return src_sl, col_sl, tile_ofs, Ttot, Epad


def _wrap_idx16(idx):
    a = idx.astype(np.int16).reshape(-1, 16).T
    return np.tile(a, (8, 1))


def _host_prep(inputs):
    x = np.asarray(inputs["x"], dtype=np.float32)
    W1 = np.asarray(inputs["W1"], dtype=np.float32)
    att_src1 = np.asarray(inputs["att_src1"], dtype=np.float32)
    att_dst1 = np.asarray(inputs["att_dst1"], dtype=np.float32)
    b1 = np.asarray(inputs["b1"], dtype=np.float32)
    W2 = np.asarray(inputs["W2"], dtype=np.float32)
    att_src2 = np.asarray(inputs["att_src2"], dtype=np.float32)
    att_dst2 = np.asarray(inputs["att_dst2"], dtype=np.float32)
    b2 = np.asarray(inputs["b2"], dtype=np.float32)
    ei = np.asarray(inputs["edge_index"])

    src_sl, col_sl, tile_ofs, Ttot, Epad = _build_partition(ei)

    bf16 = ml_dtypes.bfloat16
    fp8 = ml_dtypes.float8_e4m3fn

    # W1cat = [W1@Asrc(5) | W1@Adst(5) | W1(320)] -> node row = x @ W1cat
    W1Asrc = np.stack([W1[:, 64 * h:64 * h + 64] @ att_src1[h] for h in range(H1)], axis=1)
    W1Adst = np.stack([W1[:, 64 * h:64 * h + 64] @ att_dst1[h] for h in range(H1)], axis=1)
    W1cat = np.concatenate([W1Asrc, W1Adst, W1], axis=1).astype(bf16)  # [128, 330]

    # W2cat tiles: [128, 3, 66] = [W2 | W2@asrc2 | W2@adst2] zero-padded
    W2c = np.concatenate([W2, (W2 @ att_src2[0])[:, None],
                          (W2 @ att_dst2[0])[:, None]], axis=1)  # [320, 66]
    W2p = np.zeros((384, 66), dtype=np.float32)
    W2p[:320] = W2c
    W2cat = np.ascontiguousarray(W2p.reshape(3, 128, 66).transpose(1, 0, 2)).astype(bf16)

    xT = np.ascontiguousarray(x.T)
    shared = dict(
        xT=xT.astype(bf16),                              # [128, 10000]
        W1cat=W1cat,
        W2cat=W2cat,
        ident=np.eye(128, dtype=np.float32).astype(bf16),
        b1rep=np.broadcast_to(b1, (128, F1)).copy(),
        b2rep=np.broadcast_to(b2, (128, F2)).copy(),
    )
    # h1 table row permutation: phase A writes groups of 4 node tiles as
    # [p, i] -> row 512g + 4p + i so each partition's 4 rows are contiguous
    # (3KB runs -> 4x fewer DMA descriptors). Tail nodes keep identity rows.
    n_full = (N_NODES // 512) * 512
    nn = np.arange(N_NODES, dtype=np.int64)
    perm1 = np.where(
        nn < n_full,
        (nn // 512) * 512 + (nn % 128) * 4 + (nn % 512) // 128,
        nn)

    d = np.arange(128, dtype=np.float32)
    per_core = []
    for c in range(N_CORES):
        colf = np.ascontiguousarray(col_sl[c].reshape(Ttot, TILE_E).T)  # [128, Ttot]
        S = (colf[:, :, None] == d[None, None, :])                      # [e,t,d]
        per_core.append(dict(
            src16=_wrap_idx16(perm1[src_sl[c]]),
            src16b=_wrap_idx16(src_sl[c]),
            Sb=np.ascontiguousarray(S).astype(fp8),
            STb=np.ascontiguousarray(np.transpose(S, (2, 1, 0))).astype(fp8),
            xTc=np.ascontiguousarray(xT[:, c * NPC:(c + 1) * NPC]).astype(bf16),
        ))
    return shared, per_core, tile_ofs, Ttot, Epad


def _build_program(tile_ofs, Ttot, Epad):
    import concourse.bacc as bacc
    import concourse.mybir as mybir
    from concourse import tile

    dt = mybir.dt
    F32 = dt.float32
    BF16 = dt.bfloat16
    FP8 = dt.float8e4
    AF = mybir.ActivationFunctionType
    OP = mybir.AluOpType

    B = B_BLOCKS
    tile_ofs = [int(v) for v in tile_ofs]
    block_of_tile = np.zeros(Ttot, dtype=np.int64)
    for b in range(B):
        block_of_tile[tile_ofs[b]:tile_ofs[b + 1]] = b
    n_chunks = (Ttot + CHUNK - 1) // CHUNK
    n_node_tiles = (N_NODES + 127) // 128

    nc = bacc.Bacc("TRN2", target_bir_lowering=False, debug=False,
                   num_devices=N_CORES, num_swdge_queues=4)

    xT_d = nc.dram_tensor("xT", [F0, N_NODES], BF16, kind="ExternalInput")
    W1c_d = nc.dram_tensor("W1cat", [F0, 2 * H1 + F1], BF16, kind="ExternalInput")
    W2c_d = nc.dram_tensor("W2cat", [128, 3, F2 + 2], BF16, kind="ExternalInput")
    ident_d = nc.dram_tensor("ident", [128, 128], BF16, kind="ExternalInput")
    b1_d = nc.dram_tensor("b1rep", [128, F1], F32, kind="ExternalInput")
    b2_d = nc.dram_tensor("b2rep", [128, F2], F32, kind="ExternalInput")
    src16_d = nc.dram_tensor("src16", [128, Epad // 16], dt.int16, kind="ExternalInput")
    src16b_d = nc.dram_tensor("src16b", [128, Epad // 16], dt.int16, kind="ExternalInput")
    S_d = nc.dram_tensor("Sb", [128, Ttot, 128], FP8, kind="ExternalInput")
    ST_d = nc.dram_tensor("STb", [128, Ttot, 128], FP8, kind="ExternalInput")
    xTc_d = nc.dram_tensor("xTc", [F0, NPC], BF16, kind="ExternalInput")
    out_d = nc.dram_tensor("out", [NPC, F2], F32, kind="ExternalOutput")

    with tile.TileContext(nc) as tc:
        with (
            tc.tile_pool(name="dram", bufs=1, space="DRAM") as dram,
            tc.tile_pool(name="const", bufs=1) as cpool,
        ):
            h1tab = dram.tile([N_NODES, ROW1], BF16)
            ag_in = dram.tile([NPC, F2 + 2], BF16)
            h2pack = dram.tile([N_NODES, F2 + 2], BF16, addr_space="Shared")
            h2tab = dram.tile([N_NODES, ROW2], BF16)

            W1cs = cpool.tile([F0, 2 * H1 + F1], BF16)
            nc.sync.dma_start(W1cs[:], W1c_d[:])
            W2cs = cpool.tile([128, 3, F2 + 2], BF16)
            nc.sync.dma_start(W2cs[:], W2c_d[:])
            idents = cpool.tile([128, 128], BF16)
            nc.sync.dma_start(idents[:], ident_d[:])
            b1s = cpool.tile([128, F1], F32)
            nc.sync.dma_start(b1s[:], b1_d[:])
            b2s = cpool.tile([128, F2], F32)
            nc.sync.dma_start(b2s[:], b2_d[:])
            src16 = cpool.tile([128, Epad // 16], dt.int16)
            nc.sync.dma_start(src16[:], src16_d[:])
            src16b = cpool.tile([128, Epad // 16], dt.int16)
            nc.sync.dma_start(src16b[:], src16b_d[:])
            S_sb = cpool.tile([128, Ttot, 128], FP8)
            nc.sync.dma_start(S_sb[:], S_d[:])
            ST_sb = cpool.tile([128, Ttot, 128], FP8)
            nc.sync.dma_start(ST_sb[:], ST_d[:])
            xTcs = cpool.tile([F0, NPC], BF16)
            nc.sync.dma_start(xTcs[:], xTc_d[:])
            adst1s = cpool.tile([128, B, H1], BF16)
            adst2s = cpool.tile([128, B, 1], BF16)

            # ---- phase A: node table + own-dst adst1 -----------------------
            with (
                tc.tile_pool(name="pAx", bufs=1) as pAx,
                tc.tile_pool(name="pA", bufs=3) as pA,
                tc.tile_pool(name="psA", bufs=3, space="PSUM") as psA,
            ):
                for b in range(B):
                    pa = psA.tile([128, H1], F32, tag="pa")
                    nc.tensor.matmul(pa[:NPB, :], xTcs[:, NPB * b:NPB * (b + 1)],
                                     W1cs[:, H1:2 * H1], start=True, stop=True)
                    nc.vector.tensor_copy(adst1s[:NPB, b, :], pa[:NPB, :])
                xTs = pAx.tile([F0, N_NODES], BF16)
                XCH = 2560
                for xo in range(0, N_NODES, XCH):
                    xw = min(XCH, N_NODES - xo)
                    nc.sync.dma_start(xTs[:, xo:xo + xw], xT_d[:, xo:xo + xw])
                GRP = 4
                for nt0 in range(0, n_node_tiles, GRP):
                    gn = min(GRP, n_node_tiles - nt0)
                    row = pA.tile([128, GRP, ROW1], BF16, tag="row")
                    for i in range(gn):
                        nt = nt0 + i
                        w = min(128, N_NODES - 128 * nt)
                        ph = psA.tile([128, 2 * H1 + F1], F32, tag="ph")
                        nc.tensor.matmul(ph[:w, :], xTs[:, 128 * nt:128 * nt + w],
                                         W1cs[:], start=True, stop=True)
                        if i % 2 == 0:
                            nc.vector.tensor_copy(row[:w, i, 0:2 * H1 + F1],
                                                  ph[:w, :])
                        else:
                            nc.scalar.activation(row[:w, i, 0:2 * H1 + F1],
                                                 ph[:w, :], AF.Copy)
                    if 128 * (nt0 + gn) <= n_node_tiles * 128 and gn == GRP:
                        # permuted rows: row 512g + 4p + i; [p, i] contiguous
                        dst = h1tab[512 * (nt0 // GRP):512 * (nt0 // GRP + 1), :]
                        dst = dst.rearrange("(p i) r -> p i r", i=GRP)
                        nc.sync.dma_start(dst, row[:])
                    else:
                        for i in range(gn):
                            nt = nt0 + i
                            w = min(128, N_NODES - 128 * nt)
                            nc.sync.dma_start(
                                h1tab[128 * nt:128 * nt + w, 0:2 * H1 + F1],
                                row[:w, i, 0:2 * H1 + F1])

            # ---- layer 1 edge sweep ---------------------------------------
            with (
                tc.tile_pool(name="gbuf", bufs=12) as gbuf,
                tc.tile_pool(name="sb1", bufs=4) as sb1,
                tc.tile_pool(name="ps_es", bufs=3, space="PSUM") as ps_es,
                tc.tile_pool(name="ps_u", bufs=2, space="PSUM") as ps_u,
                tc.tile_pool(name="ps_t", bufs=1, space="PSUM") as ps_t,
                tc.tile_pool(name="epi", bufs=2) as epi,
            ):
                usp = None
                for ch in range(n_chunks):
                    t0 = ch * CHUNK
                    tn = min(CHUNK, Ttot - t0)
                    g = gbuf.tile([128, CHUNK, ROW1], BF16, tag="g")
                    nc.gpsimd.dma_gather(
                        g[:, 0:tn, :], h1tab[:],
                        src16[:, t0 * 8:t0 * 8 + tn * 8],
                        num_idxs=tn * TILE_E, num_idxs_reg=tn * TILE_E,
                        elem_size=ROW1, queue_num=ch % 4)
                    esp = ps_es.tile([128, CHUNK, H1], F32, tag="esp")
                    for tl in range(tn):
                        t = t0 + tl
                        b = int(block_of_tile[t])
                        # edp = a_dst[dst] per edge
                        nc.tensor.matmul(esp[:, tl, :], ST_sb[:NPB, t, :],
                                         adst1s[:NPB, b, :], start=True, stop=True)
                    # chunk-batched a_src add + leaky-relu + exp + exp*h
                    esl = sb1.tile([128, CHUNK, H1], F32, tag="esl", bufs=3)
                    nc.vector.tensor_tensor(esl[:, 0:tn, :], esp[:, 0:tn, :],
                                            g[:, 0:tn, 0:H1], OP.add)
                    eslp = sb1.tile([128, CHUNK, H1], F32, tag="eslp", bufs=3)
                    nc.scalar.activation(eslp[:, 0:tn, :], esl[:, 0:tn, :],
                                         AF.Prelu, alpha=NEG_SLOPE)
                    exhs = sb1.tile([128, CHUNK, H1 + F1], BF16, tag="exhs")
                    nc.scalar.activation(exhs[:, 0:tn, 0:H1], eslp[:, 0:tn, :],
                                         AF.Exp)
                    g4 = g[:, 0:tn, 2 * H1:2 * H1 + F1].rearrange(
                        "p t (h c) -> p t h c", h=H1)
                    ex4 = exhs[:, 0:tn, 0:H1].unsqueeze(3).broadcast_to(
                        (128, tn, H1, C1))
                    o4 = exhs[:, 0:tn, H1:].rearrange("p t (h c) -> p t h c", h=H1)
                    nc.vector.tensor_tensor(o4, g4, ex4, OP.mult)
                    for tl in range(tn):
                        t = t0 + tl
                        b = int(block_of_tile[t])
                        first = t == tile_ofs[b]
                        last = t == tile_ofs[b + 1] - 1
                        if first:
                            usp = ps_u.tile([128, H1 + F1], F32, tag="usp")
                        nc.tensor.matmul(usp[:, :], S_sb[:, t, :],
                                         exhs[:, tl, :], start=first, stop=last)
                        if last:
                            # epilogue: alpha normalize + bias + relu
                            rec = epi.tile([128, H1], F32, tag="rec")
                            nc.vector.tensor_scalar_add(rec[:NPB, :],
                                                        usp[:NPB, 0:H1], EPS)
                            nc.vector.reciprocal(rec[:NPB, :], rec[:NPB, :])
                            o1 = epi.tile([128, F1], F32, tag="o1")
                            u4 = usp[:NPB, H1:].rearrange("p (h c) -> p h c", h=H1)
                            r4 = rec[:NPB, :].unsqueeze(2).broadcast_to(
                                (NPB, H1, C1))
                            o14 = o1[:NPB, :].rearrange("p (h c) -> p h c", h=H1)
                            nc.vector.tensor_tensor(o14, u4, r4, OP.mult)
                            nc.vector.tensor_tensor(o1[:NPB, :], o1[:NPB, :],
                                                    b1s[:NPB, :], OP.add)
                            o1r = epi.tile([128, F1], BF16, tag="o1r")
                            nc.scalar.activation(o1r[:NPB, :], o1[:NPB, :],
                                                 AF.Relu)
                            # sink: h2 = relu(out1) @ [W2|W2a_src2|W2a_dst2]
                            h1T = epi.tile([128, 3, NPB], BF16, tag="h1T")
                            for k in range(3):
                                w3 = min(128, F1 - 128 * k)
                                tp = ps_t.tile([128, NPB], BF16, tag="tp")
                                nc.tensor.transpose(
                                    tp[:w3, :], o1r[:NPB, 128 * k:128 * k + w3],
                                    idents[:NPB, :NPB])
                                nc.vector.tensor_copy(h1T[:w3, k, :], tp[:w3, :])
                            h2ps = ps_t.tile([128, F2 + 2], F32, tag="h2ps")
                            for k in range(3):
                                w3 = min(128, F1 - 128 * k)
                                nc.tensor.matmul(h2ps[:NPB, :], h1T[:w3, k, :],
                                                 W2cs[:w3, k, :],
                                                 start=(k == 0), stop=(k == 2))
                            agrow = epi.tile([128, F2 + 2], BF16, tag="agrow")
                            nc.vector.tensor_copy(agrow[:NPB, :],
                                                  h2ps[:NPB, :])
                            nc.sync.dma_start(ag_in[NPB * b:NPB * (b + 1), :],
                                              agrow[:NPB, :])
                            nc.vector.tensor_copy(adst2s[:NPB, b, :],
                                                  h2ps[:NPB, F2 + 1:F2 + 2])

            nc.gpsimd.collective_compute(
                "AllGather", mybir.AluOpType.bypass,
                replica_groups=[list(range(N_CORES))],
                ins=[ag_in.opt()], outs=[h2pack.opt()])
            nc.sync.dma_start(h2tab[0:N_NODES // 2, 0:F2 + 2],
                              h2pack[0:N_NODES // 2, :])
            nc.scalar.dma_start(h2tab[N_NODES // 2:, 0:F2 + 2],
                                h2pack[N_NODES // 2:, :])

            # ---- layer 2 edge sweep ---------------------------------------
            with (
                tc.tile_pool(name="gbuf2", bufs=12) as gbuf2,
                tc.tile_pool(name="sb2", bufs=4) as sb2,
                tc.tile_pool(name="ps_e2", bufs=1, space="PSUM") as ps_e2,
                tc.tile_pool(name="ps_u2", bufs=2, space="PSUM") as ps_u2,
                tc.tile_pool(name="epi2", bufs=2) as epi2,
            ):
                # a_dst2 expanded to edge slots per chunk, one PSUM bank
                ep2 = ps_e2.tile([128, Ttot, 1], F32)
                usp2 = None
                for ch in range(n_chunks):
                    t0 = ch * CHUNK
                    tn = min(CHUNK, Ttot - t0)
                    for tl in range(tn):
                        t = t0 + tl
                        b = int(block_of_tile[t])
                        nc.tensor.matmul(ep2[:, t, :], ST_sb[:NPB, t, :],
                                         adst2s[:NPB, b, :], start=True, stop=True)
                    g2 = gbuf2.tile([128, CHUNK, ROW2], BF16, tag="g2")
                    nc.gpsimd.dma_gather(
                        g2[:, 0:tn, :], h2tab[:],
                        src16b[:, t0 * 8:t0 * 8 + tn * 8],
                        num_idxs=tn * TILE_E, num_idxs_reg=tn * TILE_E,
                        elem_size=ROW2, queue_num=ch % 4)
                    es2 = sb2.tile([128, CHUNK, 1], F32, tag="es2")
                    nc.vector.tensor_tensor(es2[:, 0:tn, :],
                                            g2[:, 0:tn, F2:F2 + 1],
                                            ep2[:, t0:t0 + tn, :], OP.add)
                    es2l = sb2.tile([128, CHUNK, 1], F32, tag="es2l")
                    nc.scalar.activation(es2l[:, 0:tn, :], es2[:, 0:tn, :],
                                         AF.Prelu, alpha=NEG_SLOPE)
                    exhs2 = sb2.tile([128, CHUNK, 1 + F2], BF16, tag="exhs2")
                    nc.scalar.activation(exhs2[:, 0:tn, 0:1], es2l[:, 0:tn, :],
                                         AF.Exp)
                    ex2 = exhs2[:, 0:tn, 0:1].broadcast_to((128, tn, F2))
                    nc.vector.tensor_tensor(exhs2[:, 0:tn, 1:], g2[:, 0:tn, 0:F2],
                                            ex2, OP.mult)
                    for tl in range(tn):
                        t = t0 + tl
                        b = int(block_of_tile[t])
                        first = t == tile_ofs[b]
                        last = t == tile_ofs[b + 1] - 1
                        if first:
                            usp2 = ps_u2.tile([128, 1 + F2], F32, tag="usp2")
                        nc.tensor.matmul(usp2[:, :], S_sb[:, t, :],
                                         exhs2[:, tl, :], start=first, stop=last)
                        if last:
                            rec2 = epi2.tile([128, 1], F32, tag="rec2")
                            nc.vector.tensor_scalar_add(rec2[:NPB, :],
                                                        usp2[:NPB, 0:1], EPS)
                            nc.vector.reciprocal(rec2[:NPB, :], rec2[:NPB, :])
                            o2 = epi2.tile([128, F2], F32, tag="o2")
                            nc.vector.scalar_tensor_tensor(
                                o2[:NPB, :], usp2[:NPB, 1:], rec2[:NPB, :],
                                b2s[:NPB, :], OP.mult, OP.add)
                            o2r = epi2.tile([128, F2], F32, tag="o2r")
                            nc.scalar.activation(o2r[:NPB, :], o2[:NPB, :],
                                                 AF.Relu)
                            nc.sync.dma_start(out_d[NPB * b:NPB * (b + 1), :],
                                              o2r[:NPB, :])

    nc.compile()
    return nc


def kernel(**inputs) -> np.ndarray:
    import time

    from concourse.bass_utils import run_bass_kernel_spmd

    shared, per_core, tile_ofs, Ttot, Epad = _host_prep(inputs)
    nc = _build_program(tile_ofs, Ttot, Epad)

    in_maps = []
    for c in range(N_CORES):
        m = dict(shared)
        m.update(per_core[c])
        in_maps.append(m)
    res = None
    for attempt in range(3):
        try:
            res = run_bass_kernel_spmd(nc, in_maps, list(range(N_CORES)))
            break
        except Exception:
            if attempt == 2:
                raise
            time.sleep(5)
    out = np.concatenate([res.results[c]["out"] for c in range(N_CORES)], axis=0)
    return np.ascontiguousarray(out.astype(np.float32))


# revision 28
# speedup vs baseline: 1.4722x; 1.0170x over previous
"""Trainium2 Bass kernel for the 2-layer GAT block (nn_GATblock_58282706206740).

Strategy (8 NeuronCores, SPMD, dst-sharded):
  - Edges (incl. self-loops) sharded by destination-node range: core c owns
    dst nodes [1250c, 1250(c+1)), split into 10 blocks of 125. Per-(core,
    block) edge lists padded to a common per-block tile count (max over
    cores) of 128-edge tiles, so one program serves all cores.
  - Phase A (replicated, bf16): node table rows [asrc1(5)|adst1(5)|h1(320)]
    written to DRAM (768B rows); per-edge rows arrive via gpsimd dma_gather.
  - dma_gather descriptor generation is the machine's serial bottleneck
    (~8.4 ns/idx on one Q7 core pair); gathers round-robin over 4 SWDGE
    queues so 4 desc-gens+transfers run concurrently (~4x).
  - Scores e = a_src[src]+a_dst[dst] are summed in PSUM by two accumulating
    PE matmuls per tile (a_dst expands via the fp8 ST one-hot; a_src copies
    in via an identity matmul). leaky-relu/exp run chunk-batched on the
    Scalar engine (Prelu/Exp share one activation table); exp*h is one
    chunk-batched DVE multiply in bf16.
  - Segment softmax-sum + message aggregation is one fused PE matmul per
    tile with rhs [exp | exp*h] (bf16) against the fp8 S one-hot (0/1 is
    exact in fp8; fp8 lhsT pairs with bf16 rhs, verified exact on HW).
    Softmax skips the max subtraction (scores provably small) and keeps
    the reference's +1e-16 denominator epsilon.
  - Between layers one AllGather exchanges the bf16 [h2 | a_src2] table.
"""
import sys

sys.path.insert(0, "/opt/trn_rl_repo")

import ml_dtypes
import numpy as np

N_NODES = 10000
N_CORES = 8
NPC = N_NODES // N_CORES          # 1250
B_BLOCKS = 10
NPB = NPC // B_BLOCKS             # 125
TILE_E = 128
CHUNK = 8                         # tiles per gather call (1024 idx)
EPS = 1e-16
NEG_SLOPE = 0.2
F0, F1, F2, H1, C1 = 128, 320, 64, 5, 64
ROW1 = 384                        # [asrc1(5) | adst1(5) | h1(320) | pad]
ROW2 = 128                        # [h2(64) | asrc2(1) | pad]


def _build_partition(edge_index):
    src = np.concatenate([edge_index[0].astype(np.int64),
                          np.arange(N_NODES, dtype=np.int64)])
    dst = np.concatenate([edge_index[1].astype(np.int64),
                          np.arange(N_NODES, dtype=np.int64)])
    core = dst // NPC
    block = (dst % NPC) // NPB
    col = dst % NPB

    cnt = np.zeros((N_CORES, B_BLOCKS), dtype=np.int64)
    np.add.at(cnt, (core, block), 1)
    T_b = np.ceil(cnt.max(axis=0) / TILE_E).astype(np.int64)
    tile_ofs = np.concatenate([[0], np.cumsum(T_b)])
    Ttot = int(tile_ofs[-1])
    Epad = Ttot * TILE_E

    src_sl = np.zeros((N_CORES, Epad), dtype=np.int64)
    col_sl = np.full((N_CORES, Epad), 200.0, dtype=np.float32)
    order = np.lexsort((dst, core * B_BLOCKS + block))
    s_src, s_core, s_block, s_col = src[order], core[order], block[order], col[order]
    idx = 0
    for c in range(N_CORES):
        for b in range(B_BLOCKS):
            n = int(cnt[c, b])
            base = int(tile_ofs[b]) * TILE_E
            sl = slice(idx, idx + n)
            assert np.all(s_core[sl] == c) and np.all(s_block[sl] == b)
            src_sl[c, base:base + n] = s_src[sl]
            col_sl[c, base:base + n] = s_col[sl]
            idx += n
    assert idx == len(src)
    return src_sl, col_sl, tile_ofs, Ttot, Epad


def _wrap_idx16(idx):
    a = idx.astype(np.int16).reshape(-1, 16).T
    return np.tile(a, (8, 1))


def _host_prep(inputs):
    x = np.asarray(inputs["x"], dtype=np.float32)
    W1 = np.asarray(inputs["W1"], dtype=np.float32)
    att_src1 = np.asarray(inputs["att_src1"], dtype=np.float32)
    att_dst1 = np.asarray(inputs["att_dst1"], dtype=np.float32)
    b1 = np.asarray(inputs["b1"], dtype=np.float32)
    W2 = np.asarray(inputs["W2"], dtype=np.float32)
    att_src2 = np.asarray(inputs["att_src2"], dtype=np.float32)
    att_dst2 = np.asarray(inputs["att_dst2"], dtype=np.float32)
    b2 = np.asarray(inputs["b2"], dtype=np.float32)
    ei = np.asarray(inputs["edge_index"])

    src_sl, col_sl, tile_ofs, Ttot, Epad = _build_partition(ei)

    bf16 = ml_dtypes.bfloat16
    fp8 = ml_dtypes.float8_e4m3fn

    # W1cat = [W1@Asrc(5) | W1@Adst(5) | W1(320)] -> node row = x @ W1cat
    W1Asrc = np.stack([W1[:, 64 * h:64 * h + 64] @ att_src1[h] for h in range(H1)], axis=1)
    W1Adst = np.stack([W1[:, 64 * h:64 * h + 64] @ att_dst1[h] for h in range(H1)], axis=1)
    W1cat = np.concatenate([W1Asrc, W1Adst, W1], axis=1).astype(bf16)  # [128, 330]

    # W2cat tiles: [128, 3, 66] = [W2 | W2@asrc2 | W2@adst2] zero-padded
    W2c = np.concatenate([W2, (W2 @ att_src2[0])[:, None],
                          (W2 @ att_dst2[0])[:, None]], axis=1)  # [320, 66]
    W2p = np.zeros((384, 66), dtype=np.float32)
    W2p[:320] = W2c
    W2cat = np.ascontiguousarray(W2p.reshape(3, 128, 66).transpose(1, 0, 2)).astype(bf16)

    xT = np.ascontiguousarray(x.T)
    shared = dict(
        xT=xT.astype(bf16),                              # [128, 10000]
        W1cat=W1cat,
        W2cat=W2cat,
        ident=np.eye(128, dtype=np.float32).astype(bf16),
        b1rep=np.broadcast_to(b1, (128, F1)).copy(),
        b2rep=np.broadcast_to(b2, (128, F2)).copy(),
    )
    # h1 table row permutation: phase A writes groups of 4 node tiles as
    # [p, i] -> row 512g + 4p + i so each partition's 4 rows are contiguous
    # (3KB runs -> 4x fewer DMA descriptors). Tail nodes keep identity rows.
    n_full = (N_NODES // 512) * 512
    nn = np.arange(N_NODES, dtype=np.int64)
    perm1 = np.where(
        nn < n_full,
        (nn // 512) * 512 + (nn % 128) * 4 + (nn % 512) // 128,
        nn)

    d = np.arange(128, dtype=np.float32)
    per_core = []
    for c in range(N_CORES):
        colf = np.ascontiguousarray(col_sl[c].reshape(Ttot, TILE_E).T)  # [128, Ttot]
        S = (colf[:, :, None] == d[None, None, :])                      # [e,t,d]
        per_core.append(dict(
            src16=_wrap_idx16(perm1[src_sl[c]]),
            src16b=_wrap_idx16(src_sl[c]),
            Sb=np.ascontiguousarray(S).astype(fp8),
            STb=np.ascontiguousarray(np.transpose(S, (2, 1, 0))).astype(fp8),
            xTc=np.ascontiguousarray(xT[:, c * NPC:(c + 1) * NPC]).astype(bf16),
        ))
    return shared, per_core, tile_ofs, Ttot, Epad


def _build_program(tile_ofs, Ttot, Epad):
    import concourse.bacc as bacc
    import concourse.mybir as mybir
    from concourse import tile

    dt = mybir.dt
    F32 = dt.float32
    BF16 = dt.bfloat16
    FP8 = dt.float8e4
    AF = mybir.ActivationFunctionType
    OP = mybir.AluOpType

    B = B_BLOCKS
    tile_ofs = [int(v) for v in tile_ofs]
    block_of_tile = np.zeros(Ttot, dtype=np.int64)
    for b in range(B):
        block_of_tile[tile_ofs[b]:tile_ofs[b + 1]] = b
    n_chunks = (Ttot + CHUNK - 1) // CHUNK
    n_node_tiles = (N_NODES + 127) // 128

    nc = bacc.Bacc("TRN2", target_bir_lowering=False, debug=False,
                   num_devices=N_CORES, num_swdge_queues=4)

    xT_d = nc.dram_tensor("xT", [F0, N_NODES], BF16, kind="ExternalInput")
    W1c_d = nc.dram_tensor("W1cat", [F0, 2 * H1 + F1], BF16, kind="ExternalInput")
    W2c_d = nc.dram_tensor("W2cat", [128, 3, F2 + 2], BF16, kind="ExternalInput")
    ident_d = nc.dram_tensor("ident", [128, 128], BF16, kind="ExternalInput")
    b1_d = nc.dram_tensor("b1rep", [128, F1], F32, kind="ExternalInput")
    b2_d = nc.dram_tensor("b2rep", [128, F2], F32, kind="ExternalInput")
    src16_d = nc.dram_tensor("src16", [128, Epad // 16], dt.int16, kind="ExternalInput")
    src16b_d = nc.dram_tensor("src16b", [128, Epad // 16], dt.int16, kind="ExternalInput")
    S_d = nc.dram_tensor("Sb", [128, Ttot, 128], FP8, kind="ExternalInput")
    ST_d = nc.dram_tensor("STb", [128, Ttot, 128], FP8, kind="ExternalInput")
    xTc_d = nc.dram_tensor("xTc", [F0, NPC], BF16, kind="ExternalInput")
    out_d = nc.dram_tensor("out", [NPC, F2], F32, kind="ExternalOutput")

    with tile.TileContext(nc) as tc:
        with (
            tc.tile_pool(name="dram", bufs=1, space="DRAM") as dram,
            tc.tile_pool(name="const", bufs=1) as cpool,
        ):
            h1tab = dram.tile([N_NODES, ROW1], BF16)
            ag_in = dram.tile([NPC, F2 + 2], BF16)
            h2pack = dram.tile([N_NODES, F2 + 2], BF16, addr_space="Shared")
            h2tab = dram.tile([N_NODES, ROW2], BF16)

            W1cs = cpool.tile([F0, 2 * H1 + F1], BF16)
            nc.sync.dma_start(W1cs[:], W1c_d[:])
            W2cs = cpool.tile([128, 3, F2 + 2], BF16)
            nc.sync.dma_start(W2cs[:], W2c_d[:])
            idents = cpool.tile([128, 128], BF16)
            nc.sync.dma_start(idents[:], ident_d[:])
            b1s = cpool.tile([128, F1], F32)
            nc.sync.dma_start(b1s[:], b1_d[:])
            b2s = cpool.tile([128, F2], F32)
            nc.sync.dma_start(b2s[:], b2_d[:])
            src16 = cpool.tile([128, Epad // 16], dt.int16)
            nc.sync.dma_start(src16[:], src16_d[:])
            src16b = cpool.tile([128, Epad // 16], dt.int16)
            nc.sync.dma_start(src16b[:], src16b_d[:])
            S_sb = cpool.tile([128, Ttot, 128], FP8)
            nc.sync.dma_start(S_sb[:], S_d[:])
            ST_sb = cpool.tile([128, Ttot, 128], FP8)
            nc.sync.dma_start(ST_sb[:], ST_d[:])
            xTcs = cpool.tile([F0, NPC], BF16)
            nc.sync.dma_start(xTcs[:], xTc_d[:])
            adst1s = cpool.tile([128, B, H1], BF16)
            adst2s = cpool.tile([128, B, 1], BF16)

            # ---- phase A: node table + own-dst adst1 -----------------------
            with (
                tc.tile_pool(name="pAx", bufs=1) as pAx,
                tc.tile_pool(name="pA", bufs=3) as pA,
                tc.tile_pool(name="psA", bufs=3, space="PSUM") as psA,
            ):
                for b in range(B):
                    pa = psA.tile([128, H1], F32, tag="pa")
                    nc.tensor.matmul(pa[:NPB, :], xTcs[:, NPB * b:NPB * (b + 1)],
                                     W1cs[:, H1:2 * H1], start=True, stop=True)
                    nc.vector.tensor_copy(adst1s[:NPB, b, :], pa[:NPB, :])
                xTs = pAx.tile([F0, N_NODES], BF16)
                XCH = 2560
                for xo in range(0, N_NODES, XCH):
                    xw = min(XCH, N_NODES - xo)
                    nc.sync.dma_start(xTs[:, xo:xo + xw], xT_d[:, xo:xo + xw])
                GRP = 4
                for nt0 in range(0, n_node_tiles, GRP):
                    gn = min(GRP, n_node_tiles - nt0)
                    row = pA.tile([128, GRP, ROW1], BF16, tag="row")
                    for i in range(gn):
                        nt = nt0 + i
                        w = min(128, N_NODES - 128 * nt)
                        ph = psA.tile([128, 2 * H1 + F1], F32, tag="ph")
                        nc.tensor.matmul(ph[:w, :], xTs[:, 128 * nt:128 * nt + w],
                                         W1cs[:], start=True, stop=True)
                        if i % 2 == 0:
                            nc.vector.tensor_copy(row[:w, i, 0:2 * H1 + F1],
                                                  ph[:w, :])
                        else:
                            nc.scalar.activation(row[:w, i, 0:2 * H1 + F1],
                                                 ph[:w, :], AF.Copy)
                    if 128 * (nt0 + gn) <= n_node_tiles * 128 and gn == GRP:
                        # permuted rows: row 512g + 4p + i; [p, i] contiguous
                        dst = h1tab[512 * (nt0 // GRP):512 * (nt0 // GRP + 1), :]
                        dst = dst.rearrange("(p i) r -> p i r", i=GRP)
                        nc.sync.dma_start(dst, row[:])
                    else:
                        for i in range(gn):
                            nt = nt0 + i
                            w = min(128, N_NODES - 128 * nt)
                            nc.sync.dma_start(
                                h1tab[128 * nt:128 * nt + w, 0:2 * H1 + F1],
                                row[:w, i, 0:2 * H1 + F1])

            # ---- layer 1 edge sweep ---------------------------------------
            with (
                tc.tile_pool(name="gbuf", bufs=12) as gbuf,
                tc.tile_pool(name="sb1", bufs=4) as sb1,
                tc.tile_pool(name="ps_es", bufs=3, space="PSUM") as ps_es,
                tc.tile_pool(name="ps_u", bufs=2, space="PSUM") as ps_u,
                tc.tile_pool(name="ps_t", bufs=1, space="PSUM") as ps_t,
                tc.tile_pool(name="epi", bufs=2) as epi,
            ):
                usp = None
                for ch in range(n_chunks):
                    t0 = ch * CHUNK
                    tn = min(CHUNK, Ttot - t0)
                    g = gbuf.tile([128, CHUNK, ROW1], BF16, tag="g")
                    nc.gpsimd.dma_gather(
                        g[:, 0:tn, :], h1tab[:],
                        src16[:, t0 * 8:t0 * 8 + tn * 8],
                        num_idxs=tn * TILE_E, num_idxs_reg=tn * TILE_E,
                        elem_size=ROW1, queue_num=ch % 4)
                    esp = ps_es.tile([128, CHUNK, H1], F32, tag="esp")
                    for tl in range(tn):
                        t = t0 + tl
                        b = int(block_of_tile[t])
                        # es = a_dst[dst] + a_src[src], summed in PSUM
                        nc.tensor.matmul(esp[:, tl, :], ST_sb[:NPB, t, :],
                                         adst1s[:NPB, b, :], start=True, stop=False)
                        nc.tensor.matmul(esp[:, tl, :], idents[:],
                                         g[:, tl, 0:H1], start=False, stop=True)
                    # chunk-batched leaky-relu + exp + exp*h
                    eslp = sb1.tile([128, CHUNK, H1], F32, tag="eslp", bufs=3)
                    nc.scalar.activation(eslp[:, 0:tn, :], esp[:, 0:tn, :],
                                         AF.Prelu, alpha=NEG_SLOPE)
                    exhs = sb1.tile([128, CHUNK, H1 + F1], BF16, tag="exhs")
                    nc.scalar.activation(exhs[:, 0:tn, 0:H1], eslp[:, 0:tn, :],
                                         AF.Exp)
                    g4 = g[:, 0:tn, 2 * H1:2 * H1 + F1].rearrange(
                        "p t (h c) -> p t h c", h=H1)
                    ex4 = exhs[:, 0:tn, 0:H1].unsqueeze(3).broadcast_to(
                        (128, tn, H1, C1))
                    o4 = exhs[:, 0:tn, H1:].rearrange("p t (h c) -> p t h c", h=H1)
                    nc.vector.tensor_tensor(o4, g4, ex4, OP.mult)
                    for tl in range(tn):
                        t = t0 + tl
                        b = int(block_of_tile[t])
                        first = t == tile_ofs[b]
                        last = t == tile_ofs[b + 1] - 1
                        if first:
                            usp = ps_u.tile([128, H1 + F1], F32, tag="usp")
                        nc.tensor.matmul(usp[:, :], S_sb[:, t, :],
                                         exhs[:, tl, :], start=first, stop=last)
                        if last:
                            # epilogue: alpha normalize + bias + relu
                            rec = epi.tile([128, H1], F32, tag="rec")
                            nc.vector.tensor_scalar_add(rec[:NPB, :],
                                                        usp[:NPB, 0:H1], EPS)
                            nc.vector.reciprocal(rec[:NPB, :], rec[:NPB, :])
                            o1 = epi.tile([128, F1], F32, tag="o1")
                            u4 = usp[:NPB, H1:].rearrange("p (h c) -> p h c", h=H1)
                            r4 = rec[:NPB, :].unsqueeze(2).broadcast_to(
                                (NPB, H1, C1))
                            o14 = o1[:NPB, :].rearrange("p (h c) -> p h c", h=H1)
                            nc.vector.tensor_tensor(o14, u4, r4, OP.mult)
                            nc.vector.tensor_tensor(o1[:NPB, :], o1[:NPB, :],
                                                    b1s[:NPB, :], OP.add)
                            o1r = epi.tile([128, F1], BF16, tag="o1r")
                            nc.scalar.activation(o1r[:NPB, :], o1[:NPB, :],
                                                 AF.Relu)
                            # sink: h2 = relu(out1) @ [W2|W2a_src2|W2a_dst2]
                            h1T = epi.tile([128, 3, NPB], BF16, tag="h1T")
                            for k in range(3):
                                w3 = min(128, F1 - 128 * k)
                                tp = ps_t.tile([128, NPB], BF16, tag="tp")
                                nc.tensor.transpose(
                                    tp[:w3, :], o1r[:NPB, 128 * k:128 * k + w3],
                                    idents[:NPB, :NPB])
                                nc.vector.tensor_copy(h1T[:w3, k, :], tp[:w3, :])
                            h2ps = ps_t.tile([128, F2 + 2], F32, tag="h2ps")
                            for k in range(3):
                                w3 = min(128, F1 - 128 * k)
                                nc.tensor.matmul(h2ps[:NPB, :], h1T[:w3, k, :],
                                                 W2cs[:w3, k, :],
                                                 start=(k == 0), stop=(k == 2))
                            agrow = epi.tile([128, F2 + 2], BF16, tag="agrow")
                            nc.vector.tensor_copy(agrow[:NPB, :],
                                                  h2ps[:NPB, :])
                            nc.sync.dma_start(ag_in[NPB * b:NPB * (b + 1), :],
                                              agrow[:NPB, :])
                            nc.vector.tensor_copy(adst2s[:NPB, b, :],
                                                  h2ps[:NPB, F2 + 1:F2 + 2])

            nc.gpsimd.collective_compute(
                "AllGather", mybir.AluOpType.bypass,
                replica_groups=[list(range(N_CORES))],
                ins=[ag_in.opt()], outs=[h2pack.opt()])
            nc.sync.dma_start(h2tab[0:N_NODES // 2, 0:F2 + 2],
                              h2pack[0:N_NODES // 2, :])
            nc.scalar.dma_start(h2tab[N_NODES // 2:, 0:F2 + 2],
                                h2pack[N_NODES // 2:, :])

            # ---- layer 2 edge sweep ---------------------------------------
            with (
                tc.tile_pool(name="gbuf2", bufs=12) as gbuf2,
                tc.tile_pool(name="sb2", bufs=4) as sb2,
                tc.tile_pool(name="ps_e2", bufs=1, space="PSUM") as ps_e2,
                tc.tile_pool(name="ps_u2", bufs=2, space="PSUM") as ps_u2,
                tc.tile_pool(name="epi2", bufs=2) as epi2,
            ):
                # a_dst2 expanded to edge slots per chunk, one PSUM bank
                ep2 = ps_e2.tile([128, Ttot, 1], F32)
                usp2 = None
                for ch in range(n_chunks):
                    t0 = ch * CHUNK
                    tn = min(CHUNK, Ttot - t0)
                    for tl in range(tn):
                        t = t0 + tl
                        b = int(block_of_tile[t])
                        nc.tensor.matmul(ep2[:, t, :], ST_sb[:NPB, t, :],
                                         adst2s[:NPB, b, :], start=True, stop=True)
                    g2 = gbuf2.tile([128, CHUNK, ROW2], BF16, tag="g2")
                    nc.gpsimd.dma_gather(
                        g2[:, 0:tn, :], h2tab[:],
                        src16b[:, t0 * 8:t0 * 8 + tn * 8],
                        num_idxs=tn * TILE_E, num_idxs_reg=tn * TILE_E,
                        elem_size=ROW2, queue_num=ch % 4)
                    es2 = sb2.tile([128, CHUNK, 1], F32, tag="es2")
                    nc.vector.tensor_tensor(es2[:, 0:tn, :],
                                            g2[:, 0:tn, F2:F2 + 1],
                                            ep2[:, t0:t0 + tn, :], OP.add)
                    es2l = sb2.tile([128, CHUNK, 1], F32, tag="es2l")
                    nc.scalar.activation(es2l[:, 0:tn, :], es2[:, 0:tn, :],
                                         AF.Prelu, alpha=NEG_SLOPE)
                    exhs2 = sb2.tile([128, CHUNK, 1 + F2], BF16, tag="exhs2")
                    nc.scalar.activation(exhs2[:, 0:tn, 0:1], es2l[:, 0:tn, :],
                                         AF.Exp)
                    ex2 = exhs2[:, 0:tn, 0:1].broadcast_to((128, tn, F2))
                    nc.vector.tensor_tensor(exhs2[:, 0:tn, 1:], g2[:, 0:tn, 0:F2],
                                            ex2, OP.mult)
                    for tl in range(tn):
                        t = t0 + tl
                        b = int(block_of_tile[t])
                        first = t == tile_ofs[b]
                        last = t == tile_ofs[b + 1] - 1
                        if first:
                            usp2 = ps_u2.tile([128, 1 + F2], F32, tag="usp2")
                        nc.tensor.matmul(usp2[:, :], S_sb[:, t, :],
                                         exhs2[:, tl, :], start=first, stop=last)
                        if last:
                            rec2 = epi2.tile([128, 1], F32, tag="rec2")
                            nc.vector.tensor_scalar_add(rec2[:NPB, :],
                                                        usp2[:NPB, 0:1], EPS)
                            nc.vector.reciprocal(rec2[:NPB, :], rec2[:NPB, :])
                            o2 = epi2.tile([128, F2], F32, tag="o2")
                            nc.vector.scalar_tensor_tensor(
                                o2[:NPB, :], usp2[:NPB, 1:], rec2[:NPB, :],
                                b2s[:NPB, :], OP.mult, OP.add)
                            o2r = epi2.tile([128, F2], F32, tag="o2r")
                            nc.scalar.activation(o2r[:NPB, :], o2[:NPB, :],
                                                 AF.Relu)
                            nc.sync.dma_start(out_d[NPB * b:NPB * (b + 1), :],
                                              o2r[:NPB, :])

    nc.compile()
    return nc


def kernel(**inputs) -> np.ndarray:
    import time

    from concourse.bass_utils import run_bass_kernel_spmd

    shared, per_core, tile_ofs, Ttot, Epad = _host_prep(inputs)
    nc = _build_program(tile_ofs, Ttot, Epad)

    in_maps = []
    for c in range(N_CORES):
        m = dict(shared)
        m.update(per_core[c])
        in_maps.append(m)
    res = None
    for attempt in range(3):
        try:
            res = run_bass_kernel_spmd(nc, in_maps, list(range(N_CORES)))
            break
        except Exception:
            if attempt == 2:
                raise
            time.sleep(5)
    out = np.concatenate([res.results[c]["out"] for c in range(N_CORES)], axis=0)
    return np.ascontiguousarray(out.astype(np.float32))


# revision 30
# speedup vs baseline: 1.4793x; 1.0048x over previous
"""Trainium2 Bass kernel for the 2-layer GAT block (nn_GATblock_58282706206740).

Strategy (8 NeuronCores, SPMD, dst-sharded):
  - Edges (incl. self-loops) sharded by destination-node range: core c owns
    dst nodes [1250c, 1250(c+1)), split into 10 blocks of 125. Per-(core,
    block) edge lists padded to a common per-block tile count (max over
    cores) of 128-edge tiles, so one program serves all cores.
  - Phase A (replicated, bf16): node table rows [asrc1(5)|adst1(5)|h1(320)]
    written to DRAM (768B rows); per-edge rows arrive via gpsimd dma_gather.
  - dma_gather descriptor generation is the machine's serial bottleneck
    (~8.4 ns/idx on one Q7 core pair); gathers round-robin over 4 SWDGE
    queues so 4 desc-gens+transfers run concurrently (~4x).
  - Scores e = a_src[src]+a_dst[dst] are summed in PSUM by two accumulating
    PE matmuls per tile (a_dst expands via the fp8 ST one-hot; a_src copies
    in via an identity matmul). leaky-relu/exp run chunk-batched on the
    Scalar engine (Prelu/Exp share one activation table); exp*h is one
    chunk-batched DVE multiply in bf16.
  - Segment softmax-sum + message aggregation is one fused PE matmul per
    tile with rhs [exp | exp*h] (bf16) against the fp8 S one-hot (0/1 is
    exact in fp8; fp8 lhsT pairs with bf16 rhs, verified exact on HW).
    Softmax skips the max subtraction (scores provably small) and keeps
    the reference's +1e-16 denominator epsilon.
  - Between layers one AllGather exchanges the bf16 [h2 | a_src2] table.
"""
import sys

sys.path.insert(0, "/opt/trn_rl_repo")

import ml_dtypes
import numpy as np

N_NODES = 10000
N_CORES = 8
NPC = N_NODES // N_CORES          # 1250
B_BLOCKS = 10
NPB = NPC // B_BLOCKS             # 125
TILE_E = 128
CHUNK = 8                         # tiles per gather call (1024 idx)
EPS = 1e-16
NEG_SLOPE = 0.2
F0, F1, F2, H1, C1 = 128, 320, 64, 5, 64
ROW1 = 384                        # [asrc1(5) | adst1(5) | h1(320) | pad]
ROW2 = 128                        # [h2(64) | asrc2(1) | pad]


def _build_partition(edge_index):
    src = np.concatenate([edge_index[0].astype(np.int64),
                          np.arange(N_NODES, dtype=np.int64)])
    dst = np.concatenate([edge_index[1].astype(np.int64),
                          np.arange(N_NODES, dtype=np.int64)])
    core = dst // NPC
    block = (dst % NPC) // NPB
    col = dst % NPB

    cnt = np.zeros((N_CORES, B_BLOCKS), dtype=np.int64)
    np.add.at(cnt, (core, block), 1)
    T_b = np.ceil(cnt.max(axis=0) / TILE_E).astype(np.int64)
    tile_ofs = np.concatenate([[0], np.cumsum(T_b)])
    Ttot = int(tile_ofs[-1])
    Epad = Ttot * TILE_E

    src_sl = np.zeros((N_CORES, Epad), dtype=np.int64)
    col_sl = np.full((N_CORES, Epad), 200.0, dtype=np.float32)
    order = np.lexsort((dst, core * B_BLOCKS + block))
    s_src, s_core, s_block, s_col = src[order], core[order], block[order], col[order]
    idx = 0
    for c in range(N_CORES):
        for b in range(B_BLOCKS):
            n = int(cnt[c, b])
            base = int(tile_ofs[b]) * TILE_E
            sl = slice(idx, idx + n)
            assert np.all(s_core[sl] == c) and np.all(s_block[sl] == b)
            src_sl[c, base:base + n] = s_src[sl]
            col_sl[c, base:base + n] = s_col[sl]
            idx += n
    assert idx == len(src)
    return src_sl, col_sl, tile_ofs, Ttot, Epad


def _wrap_idx16(idx):
    a = idx.astype(np.int16).reshape(-1, 16).T
    return np.tile(a, (8, 1))


def _host_prep(inputs):
    x = np.asarray(inputs["x"], dtype=np.float32)
    W1 = np.asarray(inputs["W1"], dtype=np.float32)
    att_src1 = np.asarray(inputs["att_src1"], dtype=np.float32)
    att_dst1 = np.asarray(inputs["att_dst1"], dtype=np.float32)
    b1 = np.asarray(inputs["b1"], dtype=np.float32)
    W2 = np.asarray(inputs["W2"], dtype=np.float32)
    att_src2 = np.asarray(inputs["att_src2"], dtype=np.float32)
    att_dst2 = np.asarray(inputs["att_dst2"], dtype=np.float32)
    b2 = np.asarray(inputs["b2"], dtype=np.float32)
    ei = np.asarray(inputs["edge_index"])

    src_sl, col_sl, tile_ofs, Ttot, Epad = _build_partition(ei)

    bf16 = ml_dtypes.bfloat16
    fp8 = ml_dtypes.float8_e4m3fn

    # W1cat = [W1@Asrc(5) | W1@Adst(5) | W1(320)] -> node row = x @ W1cat
    W1Asrc = np.stack([W1[:, 64 * h:64 * h + 64] @ att_src1[h] for h in range(H1)], axis=1)
    W1Adst = np.stack([W1[:, 64 * h:64 * h + 64] @ att_dst1[h] for h in range(H1)], axis=1)
    W1cat = np.concatenate([W1Asrc, W1Adst, W1], axis=1).astype(bf16)  # [128, 330]

    # W2cat tiles: [128, 3, 66] = [W2 | W2@asrc2 | W2@adst2] zero-padded
    W2c = np.concatenate([W2, (W2 @ att_src2[0])[:, None],
                          (W2 @ att_dst2[0])[:, None]], axis=1)  # [320, 66]
    W2p = np.zeros((384, 66), dtype=np.float32)
    W2p[:320] = W2c
    W2cat = np.ascontiguousarray(W2p.reshape(3, 128, 66).transpose(1, 0, 2)).astype(bf16)

    xT = np.ascontiguousarray(x.T)
    shared = dict(
        xT=xT.astype(bf16),                              # [128, 10000]
        W1cat=W1cat,
        W2cat=W2cat,
        ident=np.eye(128, dtype=np.float32).astype(bf16),
        b1rep=np.broadcast_to(b1, (128, F1)).copy(),
        b2rep=np.broadcast_to(b2, (128, F2)).copy(),
    )
    # h1 table row permutation: phase A writes groups of 4 node tiles as
    # [p, i] -> row 512g + 4p + i so each partition's 4 rows are contiguous
    # (3KB runs -> 4x fewer DMA descriptors). Tail nodes keep identity rows.
    n_full = (N_NODES // 512) * 512
    nn = np.arange(N_NODES, dtype=np.int64)
    perm1 = np.where(
        nn < n_full,
        (nn // 512) * 512 + (nn % 128) * 4 + (nn % 512) // 128,
        nn)

    d = np.arange(128, dtype=np.float32)
    per_core = []
    for c in range(N_CORES):
        colf = np.ascontiguousarray(col_sl[c].reshape(Ttot, TILE_E).T)  # [128, Ttot]
        S = (colf[:, :, None] == d[None, None, :])                      # [e,t,d]
        per_core.append(dict(
            src16=_wrap_idx16(perm1[src_sl[c]]),
            src16b=_wrap_idx16(src_sl[c]),
            Sb=np.ascontiguousarray(S).astype(fp8),
            STb=np.ascontiguousarray(np.transpose(S, (2, 1, 0))).astype(fp8),
            xTc=np.ascontiguousarray(xT[:, c * NPC:(c + 1) * NPC]).astype(bf16),
        ))
    return shared, per_core, tile_ofs, Ttot, Epad


def _build_program(tile_ofs, Ttot, Epad):
    import concourse.bacc as bacc
    import concourse.mybir as mybir
    from concourse import tile

    dt = mybir.dt
    F32 = dt.float32
    BF16 = dt.bfloat16
    FP8 = dt.float8e4
    AF = mybir.ActivationFunctionType
    OP = mybir.AluOpType

    B = B_BLOCKS
    tile_ofs = [int(v) for v in tile_ofs]
    block_of_tile = np.zeros(Ttot, dtype=np.int64)
    for b in range(B):
        block_of_tile[tile_ofs[b]:tile_ofs[b + 1]] = b
    n_chunks = (Ttot + CHUNK - 1) // CHUNK
    n_node_tiles = (N_NODES + 127) // 128

    nc = bacc.Bacc("TRN2", target_bir_lowering=False, debug=False,
                   num_devices=N_CORES, num_swdge_queues=4)

    xT_d = nc.dram_tensor("xT", [F0, N_NODES], BF16, kind="ExternalInput")
    W1c_d = nc.dram_tensor("W1cat", [F0, 2 * H1 + F1], BF16, kind="ExternalInput")
    W2c_d = nc.dram_tensor("W2cat", [128, 3, F2 + 2], BF16, kind="ExternalInput")
    ident_d = nc.dram_tensor("ident", [128, 128], BF16, kind="ExternalInput")
    b1_d = nc.dram_tensor("b1rep", [128, F1], F32, kind="ExternalInput")
    b2_d = nc.dram_tensor("b2rep", [128, F2], F32, kind="ExternalInput")
    src16_d = nc.dram_tensor("src16", [128, Epad // 16], dt.int16, kind="ExternalInput")
    src16b_d = nc.dram_tensor("src16b", [128, Epad // 16], dt.int16, kind="ExternalInput")
    S_d = nc.dram_tensor("Sb", [128, Ttot, 128], FP8, kind="ExternalInput")
    ST_d = nc.dram_tensor("STb", [128, Ttot, 128], FP8, kind="ExternalInput")
    xTc_d = nc.dram_tensor("xTc", [F0, NPC], BF16, kind="ExternalInput")
    out_d = nc.dram_tensor("out", [NPC, F2], F32, kind="ExternalOutput")

    with tile.TileContext(nc) as tc:
        with (
            tc.tile_pool(name="dram", bufs=1, space="DRAM") as dram,
            tc.tile_pool(name="const", bufs=1) as cpool,
        ):
            h1tab = dram.tile([N_NODES, ROW1], BF16)
            ag_in = dram.tile([NPC, F2 + 2], BF16)
            h2pack = dram.tile([N_NODES, F2 + 2], BF16, addr_space="Shared")
            h2tab = dram.tile([N_NODES, ROW2], BF16)

            W1cs = cpool.tile([F0, 2 * H1 + F1], BF16)
            nc.sync.dma_start(W1cs[:], W1c_d[:])
            W2cs = cpool.tile([128, 3, F2 + 2], BF16)
            nc.sync.dma_start(W2cs[:], W2c_d[:])
            idents = cpool.tile([128, 128], BF16)
            nc.sync.dma_start(idents[:], ident_d[:])
            b1s = cpool.tile([128, F1], F32)
            nc.sync.dma_start(b1s[:], b1_d[:])
            b2s = cpool.tile([128, F2], F32)
            nc.sync.dma_start(b2s[:], b2_d[:])
            src16 = cpool.tile([128, Epad // 16], dt.int16)
            nc.sync.dma_start(src16[:], src16_d[:])
            src16b = cpool.tile([128, Epad // 16], dt.int16)
            nc.sync.dma_start(src16b[:], src16b_d[:])
            S_sb = cpool.tile([128, Ttot, 128], FP8)
            nc.sync.dma_start(S_sb[:], S_d[:])
            ST_sb = cpool.tile([128, Ttot, 128], FP8)
            nc.sync.dma_start(ST_sb[:], ST_d[:])
            xTcs = cpool.tile([F0, NPC], BF16)
            nc.sync.dma_start(xTcs[:], xTc_d[:])
            adst1s = cpool.tile([128, B, H1], BF16)
            adst2s = cpool.tile([128, B, 1], BF16)

            # ---- phase A: node table + own-dst adst1 -----------------------
            with (
                tc.tile_pool(name="pAx", bufs=1) as pAx,
                tc.tile_pool(name="pA", bufs=3) as pA,
                tc.tile_pool(name="psA", bufs=3, space="PSUM") as psA,
            ):
                for b in range(B):
                    pa = psA.tile([128, H1], F32, tag="pa")
                    nc.tensor.matmul(pa[:NPB, :], xTcs[:, NPB * b:NPB * (b + 1)],
                                     W1cs[:, H1:2 * H1], start=True, stop=True)
                    nc.vector.tensor_copy(adst1s[:NPB, b, :], pa[:NPB, :])
                xTs = pAx.tile([F0, N_NODES], BF16)
                XCH = 2560
                for xo in range(0, N_NODES, XCH):
                    xw = min(XCH, N_NODES - xo)
                    nc.sync.dma_start(xTs[:, xo:xo + xw], xT_d[:, xo:xo + xw])
                GRP = 4
                for nt0 in range(0, n_node_tiles, GRP):
                    gn = min(GRP, n_node_tiles - nt0)
                    row = pA.tile([128, GRP, ROW1], BF16, tag="row")
                    for i in range(gn):
                        nt = nt0 + i
                        w = min(128, N_NODES - 128 * nt)
                        ph = psA.tile([128, 2 * H1 + F1], F32, tag="ph")
                        nc.tensor.matmul(ph[:w, :], xTs[:, 128 * nt:128 * nt + w],
                                         W1cs[:], start=True, stop=True)
                        if i % 2 == 0:
                            nc.vector.tensor_copy(row[:w, i, 0:2 * H1 + F1],
                                                  ph[:w, :])
                        else:
                            nc.scalar.activation(row[:w, i, 0:2 * H1 + F1],
                                                 ph[:w, :], AF.Copy)
                    if 128 * (nt0 + gn) <= n_node_tiles * 128 and gn == GRP:
                        # permuted rows: row 512g + 4p + i; [p, i] contiguous
                        dst = h1tab[512 * (nt0 // GRP):512 * (nt0 // GRP + 1), :]
                        dst = dst.rearrange("(p i) r -> p i r", i=GRP)
                        nc.sync.dma_start(dst, row[:])
                    else:
                        for i in range(gn):
                            nt = nt0 + i
                            w = min(128, N_NODES - 128 * nt)
                            nc.sync.dma_start(
                                h1tab[128 * nt:128 * nt + w, 0:2 * H1 + F1],
                                row[:w, i, 0:2 * H1 + F1])

            # ---- layer 1 edge sweep ---------------------------------------
            with (
                tc.tile_pool(name="gbuf", bufs=12) as gbuf,
                tc.tile_pool(name="sb1", bufs=4) as sb1,
                tc.tile_pool(name="ps_es", bufs=3, space="PSUM") as ps_es,
                tc.tile_pool(name="ps_u", bufs=2, space="PSUM") as ps_u,
                tc.tile_pool(name="ps_t", bufs=1, space="PSUM") as ps_t,
                tc.tile_pool(name="epi", bufs=2) as epi,
            ):
                def _l1_epilogue(b, usp):
                    # epilogue: alpha normalize + bias + relu
                    rec = epi.tile([128, H1], F32, tag="rec")
                    nc.vector.tensor_scalar_add(rec[:NPB, :], usp[:NPB, 0:H1],
                                                EPS)
                    nc.vector.reciprocal(rec[:NPB, :], rec[:NPB, :])
                    o1 = epi.tile([128, F1], F32, tag="o1")
                    u4 = usp[:NPB, H1:].rearrange("p (h c) -> p h c", h=H1)
                    r4 = rec[:NPB, :].unsqueeze(2).broadcast_to((NPB, H1, C1))
                    o14 = o1[:NPB, :].rearrange("p (h c) -> p h c", h=H1)
                    nc.vector.tensor_tensor(o14, u4, r4, OP.mult)
                    nc.vector.tensor_tensor(o1[:NPB, :], o1[:NPB, :],
                                            b1s[:NPB, :], OP.add)
                    o1r = epi.tile([128, F1], BF16, tag="o1r")
                    nc.scalar.activation(o1r[:NPB, :], o1[:NPB, :], AF.Relu)
                    # sink: h2 = relu(out1) @ [W2|W2a_src2|W2a_dst2]
                    h1T = epi.tile([128, 3, NPB], BF16, tag="h1T")
                    for k in range(3):
                        w3 = min(128, F1 - 128 * k)
                        tp = ps_t.tile([128, NPB], BF16, tag="tp")
                        nc.tensor.transpose(tp[:w3, :],
                                            o1r[:NPB, 128 * k:128 * k + w3],
                                            idents[:NPB, :NPB])
                        nc.vector.tensor_copy(h1T[:w3, k, :], tp[:w3, :])
                    h2ps = ps_t.tile([128, F2 + 2], F32, tag="h2ps")
                    for k in range(3):
                        w3 = min(128, F1 - 128 * k)
                        nc.tensor.matmul(h2ps[:NPB, :], h1T[:w3, k, :],
                                         W2cs[:w3, k, :],
                                         start=(k == 0), stop=(k == 2))
                    agrow = epi.tile([128, F2 + 2], BF16, tag="agrow")
                    nc.vector.tensor_copy(agrow[:NPB, :], h2ps[:NPB, :])
                    nc.sync.dma_start(ag_in[NPB * b:NPB * (b + 1), :],
                                      agrow[:NPB, :])
                    nc.vector.tensor_copy(adst2s[:NPB, b, :],
                                          h2ps[:NPB, F2 + 1:F2 + 2])

                usp = None
                pending = []
                for ch in range(n_chunks + 1):
                    if ch < n_chunks:
                        t0 = ch * CHUNK
                        tn = min(CHUNK, Ttot - t0)
                    else:
                        # flush: emit the last pending chunk's accumulates
                        t0, tn, exhs = pending.pop(0)
                        for tl in range(tn):
                            t = t0 + tl
                            b = int(block_of_tile[t])
                            first = t == tile_ofs[b]
                            last = t == tile_ofs[b + 1] - 1
                            if first:
                                usp = ps_u.tile([128, H1 + F1], F32, tag="usp")
                            nc.tensor.matmul(usp[:, :], S_sb[:, t, :],
                                             exhs[:, tl, :], start=first,
                                             stop=last)
                            if last:
                                _l1_epilogue(b, usp)
                        break
                    g = gbuf.tile([128, CHUNK, ROW1], BF16, tag="g")
                    nc.gpsimd.dma_gather(
                        g[:, 0:tn, :], h1tab[:],
                        src16[:, t0 * 8:t0 * 8 + tn * 8],
                        num_idxs=tn * TILE_E, num_idxs_reg=tn * TILE_E,
                        elem_size=ROW1, queue_num=ch % 4)
                    esp = ps_es.tile([128, CHUNK, H1], F32, tag="esp")
                    for tl in range(tn):
                        t = t0 + tl
                        b = int(block_of_tile[t])
                        # es = a_dst[dst] + a_src[src], summed in PSUM
                        nc.tensor.matmul(esp[:, tl, :], ST_sb[:NPB, t, :],
                                         adst1s[:NPB, b, :], start=True, stop=False)
                        nc.tensor.matmul(esp[:, tl, :], idents[:],
                                         g[:, tl, 0:H1], start=False, stop=True)
                    # chunk-batched leaky-relu + exp + exp*h
                    eslp = sb1.tile([128, CHUNK, H1], F32, tag="eslp", bufs=3)
                    nc.scalar.activation(eslp[:, 0:tn, :], esp[:, 0:tn, :],
                                         AF.Prelu, alpha=NEG_SLOPE)
                    exhs = sb1.tile([128, CHUNK, H1 + F1], BF16, tag="exhs")
                    nc.scalar.activation(exhs[:, 0:tn, 0:H1], eslp[:, 0:tn, :],
                                         AF.Exp)
                    g4 = g[:, 0:tn, 2 * H1:2 * H1 + F1].rearrange(
                        "p t (h c) -> p t h c", h=H1)
                    ex4 = exhs[:, 0:tn, 0:H1].unsqueeze(3).broadcast_to(
                        (128, tn, H1, C1))
                    o4 = exhs[:, 0:tn, H1:].rearrange("p t (h c) -> p t h c", h=H1)
                    nc.vector.tensor_tensor(o4, g4, ex4, OP.mult)
                    pending.append((t0, tn, exhs))
                    if ch < n_chunks - 1 and len(pending) < 2:
                        continue
                    t0, tn, exhs = pending.pop(0)
                    for tl in range(tn):
                        t = t0 + tl
                        b = int(block_of_tile[t])
                        first = t == tile_ofs[b]
                        last = t == tile_ofs[b + 1] - 1
                        if first:
                            usp = ps_u.tile([128, H1 + F1], F32, tag="usp")
                        nc.tensor.matmul(usp[:, :], S_sb[:, t, :],
                                         exhs[:, tl, :], start=first, stop=last)
                        if last:
                            _l1_epilogue(b, usp)

            nc.gpsimd.collective_compute(
                "AllGather", mybir.AluOpType.bypass,
                replica_groups=[list(range(N_CORES))],
                ins=[ag_in.opt()], outs=[h2pack.opt()])
            nc.sync.dma_start(h2tab[0:N_NODES // 2, 0:F2 + 2],
                              h2pack[0:N_NODES // 2, :])
            nc.scalar.dma_start(h2tab[N_NODES // 2:, 0:F2 + 2],
                                h2pack[N_NODES // 2:, :])

            # ---- layer 2 edge sweep ---------------------------------------
            with (
                tc.tile_pool(name="gbuf2", bufs=12) as gbuf2,
                tc.tile_pool(name="sb2", bufs=4) as sb2,
                tc.tile_pool(name="ps_e2", bufs=1, space="PSUM") as ps_e2,
                tc.tile_pool(name="ps_u2", bufs=2, space="PSUM") as ps_u2,
                tc.tile_pool(name="epi2", bufs=2) as epi2,
            ):
                # a_dst2 expanded to edge slots per chunk, one PSUM bank
                ep2 = ps_e2.tile([128, Ttot, 1], F32)
                usp2 = None
                for ch in range(n_chunks):
                    t0 = ch * CHUNK
                    tn = min(CHUNK, Ttot - t0)
                    for tl in range(tn):
                        t = t0 + tl
                        b = int(block_of_tile[t])
                        nc.tensor.matmul(ep2[:, t, :], ST_sb[:NPB, t, :],
                                         adst2s[:NPB, b, :], start=True, stop=True)
                    g2 = gbuf2.tile([128, CHUNK, ROW2], BF16, tag="g2")
                    nc.gpsimd.dma_gather(
                        g2[:, 0:tn, :], h2tab[:],
                        src16b[:, t0 * 8:t0 * 8 + tn * 8],
                        num_idxs=tn * TILE_E, num_idxs_reg=tn * TILE_E,
                        elem_size=ROW2, queue_num=ch % 4)
                    es2 = sb2.tile([128, CHUNK, 1], F32, tag="es2")
                    nc.vector.tensor_tensor(es2[:, 0:tn, :],
                                            g2[:, 0:tn, F2:F2 + 1],
                                            ep2[:, t0:t0 + tn, :], OP.add)
                    es2l = sb2.tile([128, CHUNK, 1], F32, tag="es2l")
                    nc.scalar.activation(es2l[:, 0:tn, :], es2[:, 0:tn, :],
                                         AF.Prelu, alpha=NEG_SLOPE)
                    exhs2 = sb2.tile([128, CHUNK, 1 + F2], BF16, tag="exhs2")
                    nc.scalar.activation(exhs2[:, 0:tn, 0:1], es2l[:, 0:tn, :],
                                         AF.Exp)
                    ex2 = exhs2[:, 0:tn, 0:1].broadcast_to((128, tn, F2))
                    nc.vector.tensor_tensor(exhs2[:, 0:tn, 1:], g2[:, 0:tn, 0:F2],
                                            ex2, OP.mult)
                    for tl in range(tn):
                        t = t0 + tl
                        b = int(block_of_tile[t])
                        first = t == tile_ofs[b]
                        last = t == tile_ofs[b + 1] - 1
                        if first:
                            usp2 = ps_u2.tile([128, 1 + F2], F32, tag="usp2")
                        nc.tensor.matmul(usp2[:, :], S_sb[:, t, :],
                                         exhs2[:, tl, :], start=first, stop=last)
                        if last:
                            rec2 = epi2.tile([128, 1], F32, tag="rec2")
                            nc.vector.tensor_scalar_add(rec2[:NPB, :],
                                                        usp2[:NPB, 0:1], EPS)
                            nc.vector.reciprocal(rec2[:NPB, :], rec2[:NPB, :])
                            o2 = epi2.tile([128, F2], F32, tag="o2")
                            nc.vector.scalar_tensor_tensor(
                                o2[:NPB, :], usp2[:NPB, 1:], rec2[:NPB, :],
                                b2s[:NPB, :], OP.mult, OP.add)
                            o2r = epi2.tile([128, F2], F32, tag="o2r")
                            nc.scalar.activation(o2r[:NPB, :], o2[:NPB, :],
                                                 AF.Relu)
                            nc.sync.dma_start(out_d[NPB * b:NPB * (b + 1), :],
                                              o2r[:NPB, :])

    nc.compile()
    return nc


def kernel(**inputs) -> np.ndarray:
    import time

    from concourse.bass_utils import run_bass_kernel_spmd

    shared, per_core, tile_ofs, Ttot, Epad = _host_prep(inputs)
    nc = _build_program(tile_ofs, Ttot, Epad)

    in_maps = []
    for c in range(N_CORES):
        m = dict(shared)
        m.update(per_core[c])
        in_maps.append(m)
    res = None
    for attempt in range(3):
        try:
            res = run_bass_kernel_spmd(nc, in_maps, list(range(N_CORES)))
            break
        except Exception:
            if attempt == 2:
                raise
            time.sleep(5)
    out = np.concatenate([res.results[c]["out"] for c in range(N_CORES)], axis=0)
    return np.ascontiguousarray(out.astype(np.float32))
